# revision 1
# baseline (speedup 1.0000x reference)
"""Trainium2 Bass kernel for nn_GatedLinearAttention (bidirectional GLA vision block).

Strategy
--------
Data-parallel over batch: 16 batch items -> 8 cores x 2 items. No collectives.

The chunked GLA scan is reformulated as *quadratic causal attention with global
decay* (mathematically exact):   o_t = sum_{s<=t} exp(B_t - B_s) (q_t . k_s) v_s
with B = running cumsum of log-gates, so qs = q*exp(B), ks = k*exp(-B) and the
whole scan becomes one masked matmul pair per (batch, head, direction).  The
backward direction is the same with a reverse cumsum and an anti-causal mask.
Decay totals are ~-34 in log space so exp(+-34) stays inside fp32/bf16 range.

Activations are feature-major [D, tokens] in SBUF; every projection is a
natural PE matmul and can produce outputs in either orientation.  v and the
attention output come out token-major, making per-token RMS scalars free.

ACT uses only {Sigmoid} and {Ln, Exp} LUT sets (plus universal Copy/Square):
silu(x) = x*sigmoid(x), log_sigmoid(u) = Ln(Sigmoid(u)),
rsqrt(m) = Exp(-0.5*Ln(m)).  Matmul inputs bf16, fp32 accumulation in PSUM.
"""

import os
import sys
from contextlib import ExitStack

for _p in ("/opt/trn_rl_repo", "/root/.axon_site/_ro/trn_rl_repo"):
    if os.path.isdir(_p) and _p not in sys.path:
        sys.path.insert(0, _p)

import numpy as np
import ml_dtypes

import concourse.bass as bass
import concourse.tile as tile
import concourse.mybir as mybir
from concourse.bass_utils import run_bass_kernel_spmd

f32 = mybir.dt.float32
bf16 = mybir.dt.bfloat16
AF = mybir.ActivationFunctionType
ALU = mybir.AluOpType

P = 128
NCORES = 8
B = 2               # batch items per core
L = 784             # tokens per batch item (28*28)
T = B * L           # tokens per core
D = 1024            # d_model
NH = 4
HDK = 256           # per-head key dim (2 partition tiles)
HDV = 512           # per-head value dim
GLN = 16.0
EPS = 1e-5
NT7 = 7             # batch-local token tiles (6*128 + 16)
TW = [128, 128, 128, 128, 128, 128, 16]
SW = TW
TC2 = [(0, 392), (392, 392)]              # batch-local 392-col chunks
ACH = [(0, 512), (512, 272)]              # batch-local A-phase t-chunks
DEBUG_OUT = bool(int(os.environ.get("GLA_DEBUG_OUT", "0")))


def _legalize_sync_waits(nc, max_waits=1):
    """The walrus shipped here rejects >1 semaphore wait per instruction.
    Split excess waits onto chained NOPs on the same engine right before the
    offending instruction: engines run their stream in order, so blocking
    earlier is equivalent."""
    counter = 0
    for fn in nc.m.functions:
        for blk in fn.blocks:
            insts = list(blk.instructions)
            changed = False
            out = []
            for inst in insts:
                si = inst.sync_info
                if si is not None and len(si.on_wait) > max_waits:
                    waits = list(si.on_wait)
                    keep = waits[len(waits) - max_waits:]
                    move = waits[: len(waits) - max_waits]
                    for i in range(0, len(move), max_waits):
                        chunk = move[i: i + max_waits]
                        nop = mybir.InstNoOp(
                            name=f"legalize-wait-nop-{counter}", ins=[], outs=[]
                        )
                        counter += 1
                        nop.engine = inst.engine
                        nop.sync_info = mybir.SyncInfo(on_wait=chunk, on_update=[])
                        out.append(nop)
                    inst.sync_info = mybir.SyncInfo(
                        on_wait=keep, on_update=list(si.on_update)
                    )
                    changed = True
                out.append(inst)
            if changed:
                blk.instructions = out


def _build_program():
    nc = bass.Bass()

    xpad_d = nc.dram_tensor("xpad", [8, P, B * 30 * 30], bf16, kind="ExternalInput")
    cdg_d = nc.dram_tensor("cdg", [9, 8, P, P], bf16, kind="ExternalInput")
    qkvw_d = nc.dram_tensor("qkvw", [8, P, 4096], bf16, kind="ExternalInput")
    gk1w_d = nc.dram_tensor("gk1w", [8, P, 16], bf16, kind="ExternalInput")
    gk2w_d = nc.dram_tensor("gk2w", [16, 2048], bf16, kind="ExternalInput")
    b2_d = nc.dram_tensor("b2", [16, P, 1], f32, kind="ExternalInput")
    gw_d = nc.dram_tensor("gw", [8, P, 2048], bf16, kind="ExternalInput")
    ow_d = nc.dram_tensor("ow", [16, P, 1024], bf16, kind="ExternalInput")
    masks_d = nc.dram_tensor("masks", [8, P, 512], bf16, kind="ExternalInput")
    out_d = nc.dram_tensor("out", [T, 1024], f32, kind="ExternalOutput")
    dbg = {}
    if DEBUG_OUT:
        dbg["xc"] = nc.dram_tensor("dbg_xc", [8, P, T], f32, kind="ExternalOutput")
        dbg["cs"] = nc.dram_tensor("dbg_cs", [4, P, L], f32, kind="ExternalOutput")
        dbg["qsf"] = nc.dram_tensor("dbg_qsf", [2, P, L], f32, kind="ExternalOutput")
        dbg["am"] = nc.dram_tensor("dbg_am", [P, NT7 * L], f32, kind="ExternalOutput")
        dbg["ofr"] = nc.dram_tensor("dbg_ofr", [P, NT7 * HDV], f32, kind="ExternalOutput")

    with tile.TileContext(nc) as tc:
        with ExitStack() as ctx:
            cst = ctx.enter_context(tc.tile_pool(name="cst", bufs=1))
            big = ctx.enter_context(tc.tile_pool(name="big", bufs=1))
            wts = ctx.enter_context(tc.tile_pool(name="wts", bufs=1))
            gat = ctx.enter_context(tc.tile_pool(name="gat", bufs=1))
            mid = ctx.enter_context(tc.tile_pool(name="mid", bufs=1))
            sm1 = ctx.enter_context(tc.tile_pool(name="sm1", bufs=1))
            sm2 = ctx.enter_context(tc.tile_pool(name="sm2", bufs=2))
            ps = ctx.enter_context(tc.tile_pool(name="ps", bufs=8, space="PSUM"))

            def psum(rows, cols):
                pstile = ps.tile([P, 512], f32, tag="ps", name="pstile")
                return pstile[:rows, :cols]

            # ---- constants ----
            masks = cst.tile([P, 8, 512], bf16)
            nc.sync.dma_start(out=masks, in_=masks_d.rearrange("m p t -> p m t"))
            zeros = cst.tile([P, L], f32)
            nc.vector.memset(zeros[:], 0.0)
            epst = cst.tile([P, 1], f32)
            nc.vector.memset(epst[:], EPS)

            # ---- persistent activations ----
            xc = big.tile([P, 8, T], bf16)           # conv+silu output, feature-major
            gk1o = big.tile([16, T], bf16)           # low-rank gate bottleneck
            og = big.tile([P, NT7, 2048], bf16)      # gated attn out (one batch), token-major

            # ================= Stage A: depthwise conv 3x3 + silu =================
            for ft in range(8):
                xp = gat.tile([P, B, 30, 30], bf16, tag="xp")
                nc.sync.dma_start(out=xp, in_=xpad_d[ft].rearrange("p (b h w) -> p b h w", b=B, h=30))
                cd = gat.tile([P, 9, P], bf16, tag="cd")
                nc.sync.dma_start(out=cd, in_=cdg_d[:, ft].rearrange("m p q -> p m q"))
                for bi in range(B):
                    for half in range(2):
                        pt = psum(P, 392)
                        for tap in range(9):
                            a, bb = tap // 3, tap % 3
                            rhs = xp[:, bi, a + half * 14: a + half * 14 + 14, bb: bb + 28]
                            nc.tensor.matmul(pt, cd[:, tap, :], rhs,
                                             start=(tap == 0), stop=(tap == 8))
                        sgc = sm2.tile([P, 392], f32, tag="sgc")
                        nc.scalar.activation(sgc, pt, AF.Sigmoid)
                        dst = xc[:, ft, bi * L + half * 392: bi * L + (half + 1) * 392]
                        nc.vector.tensor_mul(dst, pt, sgc)
                if DEBUG_OUT:
                    xcf = sm2.tile([P, T], f32, tag="dbgxc")
                    nc.vector.tensor_copy(xcf, xc[:, ft, :])
                    nc.sync.dma_start(out=dbg["xc"][ft], in_=xcf)

            # ================= Stage B: gk1 bottleneck [16, T] =================
            w1 = wts.tile([P, 8, 16], bf16, tag="w1")
            nc.sync.dma_start(out=w1, in_=gk1w_d.rearrange("k p c -> p k c"))
            for tc4 in range(4):
                pt = psum(16, 392)
                for kt in range(8):
                    nc.tensor.matmul(pt, w1[:, kt, :], xc[:, kt, tc4 * 392:(tc4 + 1) * 392],
                                     start=(kt == 0), stop=(kt == 7))
                nc.scalar.copy(gk1o[:, tc4 * 392:(tc4 + 1) * 392], pt)

            # ================= per (batch, head) =================
            for bi in range(B):
                for h in range(NH):
                    # ---- weights for this head ----
                    wq = gat.tile([P, 8, HDK], bf16, tag="wq")
                    nc.sync.dma_start(out=wq, in_=qkvw_d[:, :, h * HDK:(h + 1) * HDK].rearrange("k p c -> p k c"))
                    wk = gat.tile([P, 8, HDK], bf16, tag="wk")
                    nc.sync.dma_start(out=wk, in_=qkvw_d[:, :, 1024 + h * HDK: 1024 + (h + 1) * HDK].rearrange("k p c -> p k c"))
                    wv = gat.tile([P, 8, HDV], bf16, tag="wv")
                    nc.sync.dma_start(out=wv, in_=qkvw_d[:, :, 2048 + h * HDV: 2048 + (h + 1) * HDV].rearrange("k p c -> p k c"))
                    gwt = gat.tile([P, 8, HDV], bf16, tag="gw")
                    nc.sync.dma_start(out=gwt, in_=gw_d[:, :, h * HDV:(h + 1) * HDV].rearrange("k p c -> p k c"))
                    w2 = gat.tile([16, 4, P], bf16, tag="w2")
                    nc.sync.dma_start(out=w2[:, 0:2, :], in_=gk2w_d[:, h * HDK:(h + 1) * HDK].rearrange("k (c p) -> k c p", c=2))
                    nc.sync.dma_start(out=w2[:, 2:4, :], in_=gk2w_d[:, 1024 + h * HDK: 1024 + (h + 1) * HDK].rearrange("k (c p) -> k c p", c=2))
                    b2t = gat.tile([P, 4], f32, tag="b2")
                    for mi, mt in enumerate([2 * h, 2 * h + 1, 8 + 2 * h, 8 + 2 * h + 1]):
                        nc.sync.dma_start(out=b2t[:, mi: mi + 1], in_=b2_d[mt])

                    # ---- gate slab for this head: silu(xc @ g_w) token-major ----
                    gate_h = mid.tile([P, NT7, HDV], bf16, tag="gate")
                    for tt in range(NT7):
                        tw = TW[tt]
                        pt = psum(tw, HDV)
                        for kt in range(8):
                            nc.tensor.matmul(pt, xc[:, kt, bi * L + tt * P: bi * L + tt * P + tw],
                                             gwt[:, kt, :], start=(kt == 0), stop=(kt == 7))
                        gsc = sm2.tile([P, HDV], f32, tag="gsig")
                        nc.scalar.activation(gsc[:tw], pt, AF.Sigmoid)
                        nc.vector.tensor_mul(gate_h[:tw, tt, :], pt, gsc[:tw])

                    # ---- v projection (token-major) ----
                    vh = mid.tile([P, NT7, HDV], bf16, tag="vh")
                    for tt in range(NT7):
                        tw = TW[tt]
                        pt = psum(tw, HDV)
                        for kt in range(8):
                            nc.tensor.matmul(pt, xc[:, kt, bi * L + tt * P: bi * L + tt * P + tw],
                                             wv[:, kt, :], start=(kt == 0), stop=(kt == 7))
                        nc.scalar.copy(vh[:tw, tt, :], pt)

                    # ---- decays + q,k projections, per column-tile ct ----
                    qsf = mid.tile([P, 2, L], bf16, tag="qsf")
                    qsb = mid.tile([P, 2, L], bf16, tag="qsb")
                    ksf = mid.tile([P, 2, L], bf16, tag="ksf")
                    ksb = mid.tile([P, 2, L], bf16, tag="ksb")
                    for ct in range(2):
                        ets = []
                        for dr in range(2):
                            mi = dr * 2 + ct
                            t1 = sm1.tile([P, L], f32, tag="t1")
                            for tc_ in range(2):
                                o0, w0 = TC2[tc_]
                                pt = psum(P, 392)
                                nc.tensor.matmul(pt, w2[:, mi, :],
                                                 gk1o[:, bi * L + o0: bi * L + o0 + w0],
                                                 start=True, stop=True)
                                nc.scalar.activation(t1[:, o0:o0 + w0], pt, AF.Sigmoid,
                                                     bias=b2t[:, mi: mi + 1])
                            t2 = sm1.tile([P, L], f32, tag="t2")
                            nc.scalar.activation(t2, t1, AF.Ln)     # log_sigmoid(u)
                            nc.vector.tensor_tensor_scan(t1, t2, zeros, 0.0, ALU.add, ALU.add)
                            src = t1                                 # cs = cumsum(ls)
                            if dr == 1:
                                # reverse-inclusive cumsum: csr = ls - cs + total
                                nc.vector.tensor_sub(t2, t2, t1)
                                nc.vector.tensor_scalar_add(t2, t2, t1[:, L - 1: L])
                                src = t2
                            eq = sm1.tile([P, L], bf16, tag=f"eq{dr}")
                            nc.scalar.activation(eq, src, AF.Exp, scale=1.0 / GLN)
                            ek = sm1.tile([P, L], bf16, tag=f"ek{dr}")
                            nc.scalar.activation(ek, src, AF.Exp, scale=-1.0 / GLN)
                            ets.append((eq, ek))
                            if DEBUG_OUT and bi == 0 and h == 0:
                                csf = sm2.tile([P, L], f32, tag="dbgcs")
                                nc.vector.tensor_copy(csf, src)
                                nc.sync.dma_start(out=dbg["cs"][mi], in_=csf)
                        for tc_ in range(2):
                            o0, w0 = TC2[tc_]
                            sl = slice(o0, o0 + w0)
                            pt = psum(P, 392)
                            for kt in range(8):
                                nc.tensor.matmul(pt, wq[:, kt, ct * P:(ct + 1) * P],
                                                 xc[:, kt, bi * L + o0: bi * L + o0 + w0],
                                                 start=(kt == 0), stop=(kt == 7))
                            nc.vector.tensor_mul(qsf[:, ct, sl], pt, ets[0][0][:, sl])
                            nc.vector.tensor_mul(qsb[:, ct, sl], pt, ets[1][0][:, sl])
                            pt = psum(P, 392)
                            for kt in range(8):
                                nc.tensor.matmul(pt, wk[:, kt, ct * P:(ct + 1) * P],
                                                 xc[:, kt, bi * L + o0: bi * L + o0 + w0],
                                                 start=(kt == 0), stop=(kt == 7))
                            nc.vector.tensor_mul(ksf[:, ct, sl], pt, ets[0][1][:, sl])
                            nc.vector.tensor_mul(ksb[:, ct, sl], pt, ets[1][1][:, sl])
                    if DEBUG_OUT and bi == 0 and h == 0:
                        for ct in range(2):
                            qf = sm2.tile([P, L], f32, tag="dbgqs")
                            nc.vector.tensor_copy(qf, qsf[:, ct, :])
                            nc.sync.dma_start(out=dbg["qsf"][ct], in_=qf)

                    # ---- A + o per direction ----
                    ofn = None
                    for dr in range(2):
                        qs = qsf if dr == 0 else qsb
                        ks = ksf if dr == 0 else ksb
                        am = mid.tile([P, NT7, L], bf16, tag="am")
                        for j in range(2):
                            jo, jw = ACH[j]
                            for si in range(NT7):
                                d = si - 4 * j
                                if dr == 0:
                                    if si * P > jo + jw - 1:
                                        continue        # fully masked
                                    mi_ = None if d < 0 else d
                                else:
                                    if si * P + SW[si] - 1 < jo:
                                        continue
                                    mi_ = None if d >= 4 else 4 + d
                                sw = SW[si]
                                pt = psum(sw, jw)
                                for ct in range(2):
                                    nc.tensor.matmul(pt, ks[:, ct, si * P: si * P + sw],
                                                     qs[:, ct, jo: jo + jw],
                                                     start=(ct == 0), stop=(ct == 1))
                                if mi_ is None:
                                    nc.scalar.copy(am[:sw, si, jo: jo + jw], pt)
                                else:
                                    nc.vector.tensor_mul(am[:sw, si, jo: jo + jw], pt,
                                                         masks[:sw, mi_, :jw])
                        if DEBUG_OUT and bi == 0 and h == 0 and dr == 0:
                            amf = sm2.tile([P, NT7 * L], f32, tag="dbgam")
                            nc.vector.tensor_copy(amf, am.rearrange("p a b -> p (a b)"))
                            nc.sync.dma_start(out=dbg["am"], in_=amf)

                        ofr = mid.tile([P, NT7, HDV], bf16, tag=f"ofr{dr}")
                        ssq = sm1.tile([P, 8], f32, tag="ssq")
                        nc.vector.memset(ssq[:], 0.0)
                        scrap = sm1.tile([P, HDV], bf16, tag="scrap")
                        for tt in range(NT7):
                            tw = TW[tt]
                            sis = list(range(0, tt + 1) if dr == 0 else range(tt, NT7))
                            pt = psum(tw, HDV)
                            for ii, si in enumerate(sis):
                                nc.tensor.matmul(pt, am[:SW[si], si, tt * P: tt * P + tw],
                                                 vh[:SW[si], si, :],
                                                 start=(ii == 0), stop=(ii == len(sis) - 1))
                            nc.scalar.activation(scrap[:tw], pt, AF.Square,
                                                 accum_out=ssq[:tw, tt: tt + 1])
                            nc.scalar.copy(ofr[:tw, tt, :], pt)
                        # r = (ssq/512 + eps)^-1/2 = exp(-0.5 * ln(ssq/512 + eps))
                        rsl = sm1.tile([P, 8], f32, tag="rsl")
                        nc.scalar.activation(rsl, ssq, AF.Ln, scale=1.0 / HDV, bias=epst[:])
                        nc.scalar.activation(rsl, rsl, AF.Exp, scale=-0.5)
                        if dr == 0:
                            for tt in range(NT7):
                                nc.vector.tensor_scalar_mul(ofr[:TW[tt], tt, :], ofr[:TW[tt], tt, :],
                                                            rsl[:TW[tt], tt: tt + 1])
                            ofn = ofr
                            if DEBUG_OUT and bi == 0 and h == 0:
                                off = sm2.tile([P, NT7 * HDV], f32, tag="dbgof")
                                nc.vector.tensor_copy(off, ofr.rearrange("p a b -> p (a b)"))
                                nc.sync.dma_start(out=dbg["ofr"], in_=off)
                        else:
                            for tt in range(NT7):
                                tw = TW[tt]
                                nc.vector.scalar_tensor_tensor(
                                    ofr[:tw, tt, :], ofr[:tw, tt, :], rsl[:tw, tt: tt + 1],
                                    ofn[:tw, tt, :], ALU.mult, ALU.add)
                                nc.vector.tensor_mul(og[:tw, tt, h * HDV:(h + 1) * HDV],
                                                     ofr[:tw, tt, :],
                                                     gate_h[:tw, tt, :])

                # ======== Stage E for this batch: out = og @ o_w ========
                for nch in range(2):
                    owh = wts.tile([P, 16, 512], bf16, tag="owh")
                    nc.sync.dma_start(out=owh, in_=ow_d[:, :, nch * 512:(nch + 1) * 512].rearrange("j p c -> p j c"))
                    for g0 in range(0, NT7, 2):
                        tts = [tt for tt in (g0, g0 + 1) if tt < NT7]
                        ogT = gat.tile([P, 2, 16, P], bf16, tag="ogT")
                        for i, tt in enumerate(tts):
                            for jt in range(16):
                                nc.sync.dma_start_transpose(ogT[:, i, jt, :TW[tt]],
                                                            og[:TW[tt], tt, jt * P:(jt + 1) * P])
                        pts = [psum(TW[tt], 512) for tt in tts]
                        for jt in range(16):
                            for i, tt in enumerate(tts):
                                nc.tensor.matmul(pts[i], ogT[:, i, jt, :TW[tt]],
                                                 owh[:, jt, :],
                                                 start=(jt == 0), stop=(jt == 15))
                        for i, tt in enumerate(tts):
                            outs = sm2.tile([P, 512], f32, tag="outs")
                            nc.scalar.copy(outs[:TW[tt], :], pts[i])
                            nc.sync.dma_start(
                                out=out_d[bi * L + tt * P: bi * L + tt * P + TW[tt],
                                          nch * 512:(nch + 1) * 512],
                                in_=outs[:TW[tt], :])

    _legalize_sync_waits(nc)
    return nc


_CACHE = {}


def _prep_shared(conv_w, qkv_w, gk_w1, gk_w2, gk_b2, g_w, o_w, gnorm_w, lnorm_w):
    bf = ml_dtypes.bfloat16
    cdg = np.zeros((9, 8, P, P), np.float32)
    w9 = conv_w.reshape(9, D)  # taps x channels (HWIO with I=1)
    idx = np.arange(P)
    for tap in range(9):
        for ft in range(8):
            cdg[tap, ft, idx, idx] = w9[tap, ft * P:(ft + 1) * P]
    assert np.allclose(gnorm_w, lnorm_w), "kernel assumes gnorm_w == lnorm_w (fold into o_w)"
    ow_eff = o_w * np.tile(gnorm_w, NH)[:, None]
    masks = np.zeros((8, P, 512), np.float32)
    s_i = np.arange(P)[:, None]
    t_i = np.arange(512)[None, :]
    for dd in range(4):
        masks[dd] = (s_i <= t_i - P * dd)
        masks[4 + dd] = (s_i >= t_i - P * dd)
    return {
        "cdg": np.ascontiguousarray(cdg.astype(bf)),
        "qkvw": np.ascontiguousarray(qkv_w.reshape(8, P, 4096).astype(bf)),
        "gk1w": np.ascontiguousarray(gk_w1.reshape(8, P, 16).astype(bf)),
        "gk2w": np.ascontiguousarray(gk_w2.astype(bf)),
        "b2": np.ascontiguousarray(gk_b2.reshape(16, P, 1).astype(np.float32)),
        "gw": np.ascontiguousarray(g_w.reshape(8, P, 2048).astype(bf)),
        "ow": np.ascontiguousarray(ow_eff.reshape(16, P, 1024).astype(bf)),
        "masks": np.ascontiguousarray(masks.astype(bf)),
    }


def kernel(x, conv_w, qkv_w, gk_w1, gk_w2, gk_b2, g_w, g_b, o_w, gnorm_w, lnorm_w, H, W,
           _return_res=False, _trace=False):
    x = np.asarray(x, np.float32)
    assert int(H) == 28 and int(W) == 28 and x.shape == (16, L, D)
    assert np.allclose(np.asarray(g_b), 0.0), "kernel assumes g_b == 0"
    bf = ml_dtypes.bfloat16

    if "nc" not in _CACHE:
        _CACHE["nc"] = _build_program()
    nc = _CACHE["nc"]

    shared = _prep_shared(np.asarray(conv_w, np.float32), np.asarray(qkv_w, np.float32),
                          np.asarray(gk_w1, np.float32), np.asarray(gk_w2, np.float32),
                          np.asarray(gk_b2, np.float32), np.asarray(g_w, np.float32),
                          np.asarray(o_w, np.float32), np.asarray(gnorm_w, np.float32),
                          np.asarray(lnorm_w, np.float32))
    in_maps = []
    for c in range(NCORES):
        xs = x[2 * c: 2 * c + 2]                       # [2, 784, 1024]
        xt = xs.reshape(B, 28, 28, D).transpose(3, 0, 1, 2)   # [1024, 2, 28, 28]
        xpad = np.zeros((D, B, 30, 30), np.float32)
        xpad[:, :, 1:29, 1:29] = xt
        m = dict(shared)
        m["xpad"] = np.ascontiguousarray(xpad.reshape(8, P, B * 900).astype(bf))
        in_maps.append(m)

    res = run_bass_kernel_spmd(nc, in_maps, core_ids=list(range(NCORES)), trace=_trace)
    out = np.concatenate([r["out"].reshape(B, L, D) for r in res.results], axis=0)
    if _return_res:
        return out, res
    return out



# revision 9
# speedup vs baseline: 1.2167x; 1.2167x over previous
"""Trainium2 Bass kernel for nn_GatedLinearAttention (bidirectional GLA vision block).

Strategy (v2)
-------------
Data-parallel over batch: 16 batch items -> 8 cores x 2 items. No collectives.

The chunked GLA scan is reformulated as quadratic causal attention with global
decay (exact):  o_t = sum_{s<=t} exp(B_t - B_s) (q_t . k_s) v_s, B = cumsum of
log-gates.  qs = q*exp(B), ks = k*exp(-B); backward direction = anti-causal
mask with reverse cumsum.

v2 changes vs v1:
 - Stage E output projection uses PE (identity-matmul) transposes instead of
   DMA transposes (v1 spent ~556us on a serialized DMA-transpose queue).
 - Activation-table thrash removed: per batch item the schedule is grouped
   into [sigmoid-set phases] (conv/gate/decay sigmoids) and [ln-exp-set
   phases] (everything else; Copy/Square are in every set).
 - Gate is computed feature-major and fused into the stage-E transpose copy
   (one DVE mul per tile) instead of a separate token-major gate pass.
 - RMS scaling is fused into the PSUM->SBUF write of the attention output
   (per-token-tile rsqrt computed on the spot).
 - PSUM->SBUF copies moved off the scalar engine (GpSimd/Vector).
 - Weight/descriptor DMA split across the two HWDGE queues (sync + scalar).
"""

import os
import sys
from contextlib import ExitStack

for _p in ("/opt/trn_rl_repo", "/root/.axon_site/_ro/trn_rl_repo"):
    if os.path.isdir(_p) and _p not in sys.path:
        sys.path.insert(0, _p)

import numpy as np
import ml_dtypes

import concourse.bass as bass
import concourse.tile as tile
import concourse.mybir as mybir
from concourse.bass_utils import run_bass_kernel_spmd

f32 = mybir.dt.float32
bf16 = mybir.dt.bfloat16
AF = mybir.ActivationFunctionType
ALU = mybir.AluOpType

P = 128
NCORES = 8
B = 2               # batch items per core
L = 784             # tokens per batch item (28*28)
T = B * L           # tokens per core
D = 1024            # d_model
NH = 4
HDK = 256           # per-head key dim (2 partition tiles)
HDV = 512           # per-head value dim
GLN = 16.0
EPS = 1e-5
NT7 = 7             # token tiles per batch item (6*128 + 16)
TW = [128, 128, 128, 128, 128, 128, 16]
SW = TW
TC2 = [(0, 392), (392, 392)]              # 392-col token chunks
ACH = [(0, 512), (512, 272)]              # A-phase t-chunks


def _legalize_sync_waits(nc, max_waits=1):
    """The walrus shipped here rejects >1 semaphore wait per instruction.
    Split excess waits onto chained NOPs on the same engine right before the
    offending instruction: engines run their stream in order, so blocking
    earlier is equivalent."""
    counter = 0
    for fn in nc.m.functions:
        for blk in fn.blocks:
            insts = list(blk.instructions)
            changed = False
            out = []
            for inst in insts:
                si = inst.sync_info
                if si is not None and len(si.on_wait) > max_waits:
                    waits = list(si.on_wait)
                    keep = waits[len(waits) - max_waits:]
                    move = waits[: len(waits) - max_waits]
                    for i in range(0, len(move), max_waits):
                        chunk = move[i: i + max_waits]
                        nop = mybir.InstNoOp(
                            name=f"legalize-wait-nop-{counter}", ins=[], outs=[]
                        )
                        counter += 1
                        nop.engine = inst.engine
                        nop.sync_info = mybir.SyncInfo(on_wait=chunk, on_update=[])
                        out.append(nop)
                    inst.sync_info = mybir.SyncInfo(
                        on_wait=keep, on_update=list(si.on_update)
                    )
                    changed = True
                out.append(inst)
            if changed:
                blk.instructions = out


def _build_program():
    nc = bass.Bass()

    xpad_d = nc.dram_tensor("xpad", [8, P, B * 30 * 30], bf16, kind="ExternalInput")
    cdg_d = nc.dram_tensor("cdg", [9, 8, P, P], bf16, kind="ExternalInput")
    qkvw_d = nc.dram_tensor("qkvw", [8, P, 4096], bf16, kind="ExternalInput")
    gk1w_d = nc.dram_tensor("gk1w", [8, P, 16], bf16, kind="ExternalInput")
    gk2w_d = nc.dram_tensor("gk2w", [16, 2048], bf16, kind="ExternalInput")
    b2_d = nc.dram_tensor("b2", [16, P, 1], f32, kind="ExternalInput")
    gw_d = nc.dram_tensor("gw", [8, P, 2048], bf16, kind="ExternalInput")
    ow_d = nc.dram_tensor("ow", [16, P, 1024], bf16, kind="ExternalInput")
    masks_d = nc.dram_tensor("masks", [8, P, 512], bf16, kind="ExternalInput")
    ident_d = nc.dram_tensor("ident", [P, P], bf16, kind="ExternalInput")
    out_d = nc.dram_tensor("out", [T, 1024], f32, kind="ExternalOutput")

    with tile.TileContext(nc) as tc:
        with ExitStack() as ctx:
            cst = ctx.enter_context(tc.tile_pool(name="cst", bufs=1))
            big = ctx.enter_context(tc.tile_pool(name="big", bufs=1))
            wst = ctx.enter_context(tc.tile_pool(name="wst", bufs=1))
            wrk = ctx.enter_context(tc.tile_pool(name="wrk", bufs=1))
            sm = ctx.enter_context(tc.tile_pool(name="sm", bufs=1))
            est = ctx.enter_context(tc.tile_pool(name="est", bufs=1))
            ps = ctx.enter_context(tc.tile_pool(name="ps", bufs=1, space="PSUM"))

            def psum(rows, cols):
                pstile = ps.tile([P, 512], f32, tag="ps", bufs=6, name="pstile")
                return pstile[:rows, :cols]

            # ---- constants ----
            masks = cst.tile([P, 8, 512], bf16)
            nc.sync.dma_start(out=masks, in_=masks_d.rearrange("m p t -> p m t"))
            ident = cst.tile([P, P], bf16)
            nc.sync.dma_start(out=ident, in_=ident_d[:, :])
            zeros = cst.tile([P, L], bf16)
            nc.vector.memset(zeros[:], 0.0)
            epst = cst.tile([P, 1], f32)
            nc.vector.memset(epst[:], EPS)
            w1 = cst.tile([P, 8, 16], bf16)
            nc.sync.dma_start(out=w1, in_=gk1w_d.rearrange("k p c -> p k c"))
            w2 = cst.tile([16, 16, P], bf16)
            nc.sync.dma_start(out=w2, in_=gk2w_d.rearrange("k (m p) -> k m p", m=16))
            b2t = cst.tile([P, 16], f32)
            nc.sync.dma_start(out=b2t, in_=b2_d.rearrange("m p o -> p (m o)"))

            for bi in range(B):
                # persistent per-bi slabs (tag reuse across bi)
                xc = big.tile([P, 8, L], bf16, tag="xc")
                gk1o = big.tile([16, L], bf16, tag="gk1o")
                gateF = big.tile([P, 16, L], bf16, tag="gateF")
                og = big.tile([P, NT7, 2048], bf16, tag="og")
                sgf = big.tile([P, 8, L], f32, tag="sgf")   # decay sigmoids for 2 heads

                # ============ sigma-phase A: conv, gk1, gateF, decay(h0,h1) ============
                # conv 3x3 depthwise + silu (sigmoid table)
                for ft in range(8):
                    xp = wst.tile([P, 30, 30], bf16, tag="xp", bufs=2)
                    nc.sync.dma_start(
                        out=xp,
                        in_=xpad_d[ft, :, bi * 900:(bi + 1) * 900].rearrange(
                            "p (h w) -> p h w", h=30))
                    cd = wst.tile([P, 9, P], bf16, tag="cd", bufs=2)
                    nc.sync.dma_start(out=cd, in_=cdg_d[:, ft].rearrange("m p q -> p m q"))
                    for half in range(2):
                        pt = psum(P, 392)
                        for tap in range(9):
                            a, bb = tap // 3, tap % 3
                            rhs = xp[:, a + half * 14: a + half * 14 + 14, bb: bb + 28]
                            nc.tensor.matmul(pt, cd[:, tap, :], rhs,
                                             start=(tap == 0), stop=(tap == 8))
                        sgc = sm.tile([P, 392], bf16, tag="sgc", bufs=2)
                        nc.scalar.activation(sgc, pt, AF.Sigmoid)
                        nc.vector.tensor_mul(xc[:, ft, half * 392:(half + 1) * 392], pt, sgc)

                # gk1 bottleneck [16, L]
                for tc2 in range(2):
                    o0, w0 = TC2[tc2]
                    pt = psum(16, w0)
                    for kt in range(8):
                        nc.tensor.matmul(pt, w1[:, kt, :], xc[:, kt, o0:o0 + w0],
                                         start=(kt == 0), stop=(kt == 7))
                    nc.scalar.copy(gk1o[:, o0:o0 + w0], pt)

                # gate, feature-major: gateF[jt, t] = silu(gw^T xc)
                for jt in range(16):
                    gwj = wst.tile([P, 8, P], bf16, tag="gwj", bufs=2)
                    nc.scalar.dma_start(
                        out=gwj,
                        in_=gw_d[:, :, jt * P:(jt + 1) * P].rearrange("k p c -> p k c"))
                    for tc2 in range(2):
                        o0, w0 = TC2[tc2]
                        pt = psum(P, w0)
                        for kt in range(8):
                            nc.tensor.matmul(pt, gwj[:, kt, :], xc[:, kt, o0:o0 + w0],
                                             start=(kt == 0), stop=(kt == 7))
                        sgc = sm.tile([P, 392], bf16, tag="sgc", bufs=2)
                        nc.scalar.activation(sgc[:, :w0], pt, AF.Sigmoid)
                        nc.vector.tensor_mul(gateF[:, jt, o0:o0 + w0], pt, sgc[:, :w0])

                for hh in range(2):           # two half-rounds: heads (0,1) then (2,3)
                    # ---- decay-u sigmoids for this pair of heads ----
                    if hh == 1:
                        sgf = big.tile([P, 8, L], f32, tag="sgf")
                    for hl in range(2):
                        h = hh * 2 + hl
                        for dr in range(2):
                            for ct in range(2):
                                mi_g = dr * 8 + h * 2 + ct
                                slot = hl * 4 + dr * 2 + ct
                                for tc2 in range(2):
                                    o0, w0 = TC2[tc2]
                                    pt = psum(P, w0)
                                    nc.tensor.matmul(pt, w2[:, mi_g, :],
                                                     gk1o[:, o0:o0 + w0],
                                                     start=True, stop=True)
                                    nc.scalar.activation(
                                        sgf[:, slot, o0:o0 + w0], pt, AF.Sigmoid,
                                        bias=b2t[:, mi_g: mi_g + 1])

                    # ---- ln/exp phase: attention for heads hh*2, hh*2+1 ----
                    for hl in range(2):
                        h = hh * 2 + hl
                        wqkv = wst.tile([P, 8, 1024], bf16, tag="wqkv", bufs=2)
                        nc.scalar.dma_start(
                            out=wqkv[:, :, 0:256],
                            in_=qkvw_d[:, :, h * HDK:(h + 1) * HDK].rearrange("k p c -> p k c"))
                        nc.scalar.dma_start(
                            out=wqkv[:, :, 256:512],
                            in_=qkvw_d[:, :, 1024 + h * HDK: 1024 + (h + 1) * HDK].rearrange("k p c -> p k c"))
                        nc.scalar.dma_start(
                            out=wqkv[:, :, 512:1024],
                            in_=qkvw_d[:, :, 2048 + h * HDV: 2048 + (h + 1) * HDV].rearrange("k p c -> p k c"))

                        qsf = wrk.tile([P, 2, L], bf16, tag="qsf")
                        qsb = wrk.tile([P, 2, L], bf16, tag="qsb")
                        ksf = wrk.tile([P, 2, L], bf16, tag="ksf")
                        ksb = wrk.tile([P, 2, L], bf16, tag="ksb")
                        for ct in range(2):
                            # decays: fwd cs in t1; bwd reverse-inclusive cs in tl
                            t1 = wrk.tile([P, L], f32, tag="t1")
                            t2 = wrk.tile([P, L], f32, tag="t2")
                            tl = wrk.tile([P, L], f32, tag="tl")
                            nc.scalar.activation(tl, sgf[:, hl * 4 + ct, :], AF.Ln)
                            nc.vector.tensor_tensor_scan(t1, tl, zeros, 0.0,
                                                         ALU.add, ALU.add)
                            nc.scalar.activation(tl, sgf[:, hl * 4 + 2 + ct, :], AF.Ln)
                            nc.vector.tensor_tensor_scan(t2, tl, zeros, 0.0,
                                                         ALU.add, ALU.add)
                            # reverse-inclusive cumsum: ls - cs + total  (into tl)
                            nc.gpsimd.tensor_sub(tl, tl, t2)
                            nc.gpsimd.tensor_scalar_add(tl, tl, t2[:, L - 1: L])
                            eqf = wrk.tile([P, L], bf16, tag="eqf")
                            ekf = wrk.tile([P, L], bf16, tag="ekf")
                            eqb = wrk.tile([P, L], bf16, tag="eqb")
                            ekb = wrk.tile([P, L], bf16, tag="ekb")
                            nc.scalar.activation(eqf, t1, AF.Exp, scale=1.0 / GLN)
                            nc.scalar.activation(ekf, t1, AF.Exp, scale=-1.0 / GLN)
                            nc.scalar.activation(eqb, tl, AF.Exp, scale=1.0 / GLN)
                            nc.scalar.activation(ekb, tl, AF.Exp, scale=-1.0 / GLN)
                            for tc2 in range(2):
                                o0, w0 = TC2[tc2]
                                sl = slice(o0, o0 + w0)
                                pt = psum(P, w0)
                                for kt in range(8):
                                    nc.tensor.matmul(pt, wqkv[:, kt, ct * P:(ct + 1) * P],
                                                     xc[:, kt, o0:o0 + w0],
                                                     start=(kt == 0), stop=(kt == 7))
                                nc.vector.tensor_mul(qsf[:, ct, sl], pt, eqf[:, sl])
                                nc.vector.tensor_mul(qsb[:, ct, sl], pt, eqb[:, sl])
                                pt = psum(P, w0)
                                for kt in range(8):
                                    nc.tensor.matmul(pt, wqkv[:, kt, 256 + ct * P: 256 + (ct + 1) * P],
                                                     xc[:, kt, o0:o0 + w0],
                                                     start=(kt == 0), stop=(kt == 7))
                                nc.vector.tensor_mul(ksf[:, ct, sl], pt, ekf[:, sl])
                                nc.vector.tensor_mul(ksb[:, ct, sl], pt, ekb[:, sl])

                        # v projection (token-major)
                        vh = wrk.tile([P, NT7, HDV], bf16, tag="vh")
                        for tt in range(NT7):
                            tw = TW[tt]
                            pt = psum(tw, HDV)
                            for kt in range(8):
                                nc.tensor.matmul(pt, xc[:, kt, tt * P: tt * P + tw],
                                                 wqkv[:, kt, 512:1024],
                                                 start=(kt == 0), stop=(kt == 7))
                            nc.scalar.copy(vh[:tw, tt, :], pt)

                        # A + o per direction
                        for dr in range(2):
                            qs = qsf if dr == 0 else qsb
                            ks = ksf if dr == 0 else ksb
                            am = wrk.tile([P, NT7, L], bf16, tag="am")
                            for j in range(2):
                                jo, jw = ACH[j]
                                for si in range(NT7):
                                    d = si - 4 * j
                                    if dr == 0:
                                        if si * P > jo + jw - 1:
                                            continue        # fully masked
                                        mi_ = None if d < 0 else d
                                    else:
                                        if si * P + SW[si] - 1 < jo:
                                            continue
                                        mi_ = None if d >= 4 else 4 + d
                                    sw = SW[si]
                                    pt = psum(sw, jw)
                                    for ct in range(2):
                                        nc.tensor.matmul(pt, ks[:, ct, si * P: si * P + sw],
                                                         qs[:, ct, jo: jo + jw],
                                                         start=(ct == 0), stop=(ct == 1))
                                    if mi_ is None:
                                        nc.vector.tensor_copy(am[:sw, si, jo: jo + jw], pt)
                                    else:
                                        nc.vector.tensor_mul(am[:sw, si, jo: jo + jw], pt,
                                                             masks[:sw, mi_, :jw])

                            ssq = wrk.tile([P, 8], f32, tag="ssq", bufs=2)
                            nc.vector.memset(ssq[:], 0.0)
                            scrap = wrk.tile([P, HDV], bf16, tag="scrap")
                            for tt in range(NT7):
                                tw = TW[tt]
                                sis = list(range(0, tt + 1) if dr == 0 else range(tt, NT7))
                                pt = psum(tw, HDV)
                                for ii, si in enumerate(sis):
                                    nc.tensor.matmul(pt, am[:SW[si], si, tt * P: tt * P + tw],
                                                     vh[:SW[si], si, :],
                                                     start=(ii == 0), stop=(ii == len(sis) - 1))
                                nc.scalar.activation(scrap[:tw], pt, AF.Square,
                                                     accum_out=ssq[:tw, tt: tt + 1])
                                # rsl = (ssq/512 + eps)^-1/2 = exp(-0.5*ln(...))
                                rsl = wrk.tile([P, 1], f32, tag="rsl", bufs=2)
                                nc.scalar.activation(rsl[:tw], ssq[:tw, tt: tt + 1],
                                                     AF.Ln, scale=1.0 / HDV, bias=epst[:tw])
                                nc.scalar.activation(rsl[:tw], rsl[:tw], AF.Exp, scale=-0.5)
                                oslc = og[:tw, tt, h * HDV:(h + 1) * HDV]
                                if dr == 0:
                                    nc.vector.tensor_scalar_mul(oslc, pt, rsl[:tw])
                                else:
                                    nc.vector.scalar_tensor_tensor(
                                        oslc, pt, rsl[:tw], oslc, ALU.mult, ALU.add)

                # ============ stage E: out = (ogT * gateF)^T-proj ============
                for nch in range(2):
                    owS = wst.tile([P, 16, 512], bf16, tag="wqkv", bufs=2, name="owS")
                    nc.sync.dma_start(
                        out=owS,
                        in_=ow_d[:, :, nch * 512:(nch + 1) * 512].rearrange("j p c -> p j c"))
                    for tt in range(NT7):
                        tw = TW[tt]
                        ogT = est.tile([P, 16, P], bf16, tag="ogT")
                        for g in range(4):
                            ptT = ps.tile([P, 4, P], bf16, tag="psT", bufs=2, name="ptT")
                            for i in range(4):
                                jt = g * 4 + i
                                nc.tensor.transpose(ptT[:, i, :tw],
                                                    og[:tw, tt, jt * P:(jt + 1) * P],
                                                    ident[:tw, :tw])
                            nc.vector.tensor_mul(
                                ogT[:, g * 4:(g + 1) * 4, :tw],
                                ptT[:, :, :tw],
                                gateF[:, g * 4:(g + 1) * 4, tt * P: tt * P + tw])
                        pt = psum(tw, 512)
                        for jt in range(16):
                            nc.tensor.matmul(pt, ogT[:, jt, :tw], owS[:, jt, :],
                                             start=(jt == 0), stop=(jt == 15))
                        outs = sm.tile([P, 512], f32, tag="outs", bufs=2)
                        nc.vector.tensor_copy(outs[:tw], pt)
                        nc.sync.dma_start(
                            out=out_d[bi * L + tt * P: bi * L + tt * P + tw,
                                      nch * 512:(nch + 1) * 512],
                            in_=outs[:tw, :])

    _legalize_sync_waits(nc)
    return nc


_CACHE = {}


def _prep_shared(conv_w, qkv_w, gk_w1, gk_w2, gk_b2, g_w, o_w, gnorm_w, lnorm_w):
    bf = ml_dtypes.bfloat16
    cdg = np.zeros((9, 8, P, P), np.float32)
    w9 = conv_w.reshape(9, D)  # taps x channels (HWIO with I=1)
    idx = np.arange(P)
    for tap in range(9):
        for ft in range(8):
            cdg[tap, ft, idx, idx] = w9[tap, ft * P:(ft + 1) * P]
    assert np.allclose(gnorm_w, lnorm_w), "kernel assumes gnorm_w == lnorm_w (fold into o_w)"
    ow_eff = o_w * np.tile(gnorm_w, NH)[:, None]
    masks = np.zeros((8, P, 512), np.float32)
    s_i = np.arange(P)[:, None]
    t_i = np.arange(512)[None, :]
    for dd in range(4):
        masks[dd] = (s_i <= t_i - P * dd)
        masks[4 + dd] = (s_i >= t_i - P * dd)
    return {
        "cdg": np.ascontiguousarray(cdg.astype(bf)),
        "qkvw": np.ascontiguousarray(qkv_w.reshape(8, P, 4096).astype(bf)),
        "gk1w": np.ascontiguousarray(gk_w1.reshape(8, P, 16).astype(bf)),
        "gk2w": np.ascontiguousarray(gk_w2.astype(bf)),
        "b2": np.ascontiguousarray(gk_b2.reshape(16, P, 1).astype(np.float32)),
        "gw": np.ascontiguousarray(g_w.reshape(8, P, 2048).astype(bf)),
        "ow": np.ascontiguousarray(ow_eff.reshape(16, P, 1024).astype(bf)),
        "masks": np.ascontiguousarray(masks.astype(bf)),
        "ident": np.ascontiguousarray(np.eye(P, dtype=np.float32).astype(bf)),
    }


def kernel(x, conv_w, qkv_w, gk_w1, gk_w2, gk_b2, g_w, g_b, o_w, gnorm_w, lnorm_w, H, W,
           _return_res=False, _trace=False):
    x = np.asarray(x, np.float32)
    assert int(H) == 28 and int(W) == 28 and x.shape == (16, L, D)
    assert np.allclose(np.asarray(g_b), 0.0), "kernel assumes g_b == 0"
    bf = ml_dtypes.bfloat16

    if "nc" not in _CACHE:
        _CACHE["nc"] = _build_program()
    nc = _CACHE["nc"]

    shared = _prep_shared(np.asarray(conv_w, np.float32), np.asarray(qkv_w, np.float32),
                          np.asarray(gk_w1, np.float32), np.asarray(gk_w2, np.float32),
                          np.asarray(gk_b2, np.float32), np.asarray(g_w, np.float32),
                          np.asarray(o_w, np.float32), np.asarray(gnorm_w, np.float32),
                          np.asarray(lnorm_w, np.float32))
    in_maps = []
    for c in range(NCORES):
        xs = x[2 * c: 2 * c + 2]                       # [2, 784, 1024]
        xt = xs.reshape(B, 28, 28, D).transpose(3, 0, 1, 2)   # [1024, 2, 28, 28]
        xpad = np.zeros((D, B, 30, 30), np.float32)
        xpad[:, :, 1:29, 1:29] = xt
        m = dict(shared)
        m["xpad"] = np.ascontiguousarray(xpad.reshape(8, P, B * 900).astype(bf))
        in_maps.append(m)

    res = run_bass_kernel_spmd(nc, in_maps, core_ids=list(range(NCORES)), trace=_trace)
    out = np.concatenate([r["out"].reshape(B, L, D) for r in res.results], axis=0)
    if _return_res:
        return out, res
    return out


# revision 10
# speedup vs baseline: 1.4360x; 1.1802x over previous
"""Trainium2 Bass kernel for nn_GatedLinearAttention (bidirectional GLA vision block).

Strategy (v2)
-------------
Data-parallel over batch: 16 batch items -> 8 cores x 2 items. No collectives.

The chunked GLA scan is reformulated as quadratic causal attention with global
decay (exact):  o_t = sum_{s<=t} exp(B_t - B_s) (q_t . k_s) v_s, B = cumsum of
log-gates.  qs = q*exp(B), ks = k*exp(-B); backward direction = anti-causal
mask with reverse cumsum.

v2 changes vs v1:
 - Stage E output projection uses PE (identity-matmul) transposes instead of
   DMA transposes (v1 spent ~556us on a serialized DMA-transpose queue).
 - Activation-table thrash removed: per batch item the schedule is grouped
   into [sigmoid-set phases] (conv/gate/decay sigmoids) and [ln-exp-set
   phases] (everything else; Copy/Square are in every set).
 - Gate is computed feature-major and fused into the stage-E transpose copy
   (one DVE mul per tile) instead of a separate token-major gate pass.
 - RMS scaling is fused into the PSUM->SBUF write of the attention output
   (per-token-tile rsqrt computed on the spot).
 - PSUM->SBUF copies moved off the scalar engine (GpSimd/Vector).
 - Weight/descriptor DMA split across the two HWDGE queues (sync + scalar).
"""

import os
import sys
from contextlib import ExitStack

for _p in ("/opt/trn_rl_repo", "/root/.axon_site/_ro/trn_rl_repo"):
    if os.path.isdir(_p) and _p not in sys.path:
        sys.path.insert(0, _p)

import numpy as np
import ml_dtypes

import concourse.bass as bass
import concourse.tile as tile
import concourse.mybir as mybir
from concourse.bass_utils import run_bass_kernel_spmd

f32 = mybir.dt.float32
bf16 = mybir.dt.bfloat16
AF = mybir.ActivationFunctionType
ALU = mybir.AluOpType

P = 128
NCORES = 8
B = 2               # batch items per core
L = 784             # tokens per batch item (28*28)
T = B * L           # tokens per core
D = 1024            # d_model
NH = 4
HDK = 256           # per-head key dim (2 partition tiles)
HDV = 512           # per-head value dim
GLN = 16.0
EPS = 1e-5
NT7 = 7             # token tiles per batch item (6*128 + 16)
TW = [128, 128, 128, 128, 128, 128, 16]
SW = TW
TC2 = [(0, 392), (392, 392)]              # 392-col token chunks
ACH = [(0, 512), (512, 272)]              # A-phase t-chunks


def _legalize_sync_waits(nc, max_waits=1):
    """The walrus shipped here rejects >1 semaphore wait per instruction.
    Split excess waits onto chained NOPs on the same engine right before the
    offending instruction: engines run their stream in order, so blocking
    earlier is equivalent."""
    counter = 0
    for fn in nc.m.functions:
        for blk in fn.blocks:
            insts = list(blk.instructions)
            changed = False
            out = []
            for inst in insts:
                si = inst.sync_info
                if si is not None and len(si.on_wait) > max_waits:
                    waits = list(si.on_wait)
                    keep = waits[len(waits) - max_waits:]
                    move = waits[: len(waits) - max_waits]
                    for i in range(0, len(move), max_waits):
                        chunk = move[i: i + max_waits]
                        nop = mybir.InstNoOp(
                            name=f"legalize-wait-nop-{counter}", ins=[], outs=[]
                        )
                        counter += 1
                        nop.engine = inst.engine
                        nop.sync_info = mybir.SyncInfo(on_wait=chunk, on_update=[])
                        out.append(nop)
                    inst.sync_info = mybir.SyncInfo(
                        on_wait=keep, on_update=list(si.on_update)
                    )
                    changed = True
                out.append(inst)
            if changed:
                blk.instructions = out


def _build_program():
    nc = bass.Bass()

    xpad_d = nc.dram_tensor("xpad", [8, P, B * 30 * 30], bf16, kind="ExternalInput")
    cdg_d = nc.dram_tensor("cdg", [9, 8, P, P], bf16, kind="ExternalInput")
    qkvw_d = nc.dram_tensor("qkvw", [8, P, 4096], bf16, kind="ExternalInput")
    gk1w_d = nc.dram_tensor("gk1w", [8, P, 16], bf16, kind="ExternalInput")
    gk2w_d = nc.dram_tensor("gk2w", [16, 2048], bf16, kind="ExternalInput")
    b2_d = nc.dram_tensor("b2", [16, P, 1], f32, kind="ExternalInput")
    gw_d = nc.dram_tensor("gw", [8, P, 2048], bf16, kind="ExternalInput")
    ow_d = nc.dram_tensor("ow", [16, P, 1024], bf16, kind="ExternalInput")
    masks_d = nc.dram_tensor("masks", [8, P, 512], bf16, kind="ExternalInput")
    ident_d = nc.dram_tensor("ident", [P, P], bf16, kind="ExternalInput")
    out_d = nc.dram_tensor("out", [T, 1024], f32, kind="ExternalOutput")

    with tile.TileContext(nc) as tc:
        with ExitStack() as ctx:
            cst = ctx.enter_context(tc.tile_pool(name="cst", bufs=1))
            big = ctx.enter_context(tc.tile_pool(name="big", bufs=1))
            wst = ctx.enter_context(tc.tile_pool(name="wst", bufs=1))
            wrk = ctx.enter_context(tc.tile_pool(name="wrk", bufs=1))
            sm = ctx.enter_context(tc.tile_pool(name="sm", bufs=1))
            est = ctx.enter_context(tc.tile_pool(name="est", bufs=1))
            ps = ctx.enter_context(tc.tile_pool(name="ps", bufs=1, space="PSUM"))

            def psum(rows, cols):
                pstile = ps.tile([P, 512], f32, tag="ps", bufs=6, name="pstile")
                return pstile[:rows, :cols]

            # ---- constants ----
            masks = cst.tile([P, 8, 512], bf16)
            nc.sync.dma_start(out=masks, in_=masks_d.rearrange("m p t -> p m t"))
            ident = cst.tile([P, P], bf16)
            nc.sync.dma_start(out=ident, in_=ident_d[:, :])
            zeros = cst.tile([P, L], bf16)
            nc.vector.memset(zeros[:], 0.0)
            epst = cst.tile([P, 1], f32)
            nc.vector.memset(epst[:], EPS)
            w1 = cst.tile([P, 8, 16], bf16)
            nc.sync.dma_start(out=w1, in_=gk1w_d.rearrange("k p c -> p k c"))
            w2 = cst.tile([16, 16, P], bf16)
            nc.sync.dma_start(out=w2, in_=gk2w_d.rearrange("k (m p) -> k m p", m=16))
            b2t = cst.tile([P, 16], f32)
            nc.sync.dma_start(out=b2t, in_=b2_d.rearrange("m p o -> p (m o)"))

            for bi in range(B):
                # persistent per-bi slabs (tag reuse across bi)
                xc = big.tile([P, 8, L], bf16, tag="xc")
                gk1o = big.tile([16, L], bf16, tag="gk1o")
                gateF = big.tile([P, 16, L], bf16, tag="gateF")
                og = big.tile([P, NT7, 2048], bf16, tag="og")
                sgf = big.tile([P, 8, L], f32, tag="sgf")   # decay sigmoids for 2 heads

                # ============ sigma-phase A: conv, gk1, gateF, decay(h0,h1) ============
                # conv 3x3 depthwise + silu (sigmoid table)
                for ft in range(8):
                    xp = wst.tile([P, 30, 30], bf16, tag="xp", bufs=2)
                    nc.sync.dma_start(
                        out=xp,
                        in_=xpad_d[ft, :, bi * 900:(bi + 1) * 900].rearrange(
                            "p (h w) -> p h w", h=30))
                    cd = wst.tile([P, 9, P], bf16, tag="cd", bufs=2)
                    nc.sync.dma_start(out=cd, in_=cdg_d[:, ft].rearrange("m p q -> p m q"))
                    for half in range(2):
                        pt = psum(P, 392)
                        for tap in range(9):
                            a, bb = tap // 3, tap % 3
                            rhs = xp[:, a + half * 14: a + half * 14 + 14, bb: bb + 28]
                            nc.tensor.matmul(pt, cd[:, tap, :], rhs,
                                             start=(tap == 0), stop=(tap == 8))
                        sgc = sm.tile([P, 392], bf16, tag="sgc", bufs=2)
                        nc.scalar.activation(sgc, pt, AF.Sigmoid)
                        nc.vector.tensor_mul(xc[:, ft, half * 392:(half + 1) * 392], pt, sgc)

                # gk1 bottleneck [16, L]
                for tc2 in range(2):
                    o0, w0 = TC2[tc2]
                    pt = psum(16, w0)
                    for kt in range(8):
                        nc.tensor.matmul(pt, w1[:, kt, :], xc[:, kt, o0:o0 + w0],
                                         start=(kt == 0), stop=(kt == 7))
                    nc.scalar.copy(gk1o[:, o0:o0 + w0], pt)

                # gate, feature-major: gateF[jt, t] = silu(gw^T xc)
                for jt in range(16):
                    gwj = wst.tile([P, 8, P], bf16, tag="gwj", bufs=2)
                    nc.scalar.dma_start(
                        out=gwj,
                        in_=gw_d[:, :, jt * P:(jt + 1) * P].rearrange("k p c -> p k c"))
                    for tc2 in range(2):
                        o0, w0 = TC2[tc2]
                        pt = psum(P, w0)
                        for kt in range(8):
                            nc.tensor.matmul(pt, gwj[:, kt, :], xc[:, kt, o0:o0 + w0],
                                             start=(kt == 0), stop=(kt == 7))
                        sgc = sm.tile([P, 392], bf16, tag="sgc", bufs=2)
                        nc.scalar.activation(sgc[:, :w0], pt, AF.Sigmoid)
                        nc.vector.tensor_mul(gateF[:, jt, o0:o0 + w0], pt, sgc[:, :w0])

                for hh in range(2):           # two half-rounds: heads (0,1) then (2,3)
                    # ---- decay-u sigmoids for this pair of heads ----
                    if hh == 1:
                        sgf = big.tile([P, 8, L], f32, tag="sgf")
                    for hl in range(2):
                        h = hh * 2 + hl
                        for dr in range(2):
                            for ct in range(2):
                                mi_g = dr * 8 + h * 2 + ct
                                slot = hl * 4 + dr * 2 + ct
                                for tc2 in range(2):
                                    o0, w0 = TC2[tc2]
                                    pt = psum(P, w0)
                                    nc.tensor.matmul(pt, w2[:, mi_g, :],
                                                     gk1o[:, o0:o0 + w0],
                                                     start=True, stop=True)
                                    nc.scalar.activation(
                                        sgf[:, slot, o0:o0 + w0], pt, AF.Sigmoid,
                                        bias=b2t[:, mi_g: mi_g + 1])

                    # ---- ln/exp phase: attention for heads hh*2, hh*2+1 ----
                    for hl in range(2):
                        h = hh * 2 + hl
                        wqkv = wst.tile([P, 8, 1024], bf16, tag="wqkv", bufs=2)
                        nc.scalar.dma_start(
                            out=wqkv[:, :, 0:256],
                            in_=qkvw_d[:, :, h * HDK:(h + 1) * HDK].rearrange("k p c -> p k c"))
                        nc.scalar.dma_start(
                            out=wqkv[:, :, 256:512],
                            in_=qkvw_d[:, :, 1024 + h * HDK: 1024 + (h + 1) * HDK].rearrange("k p c -> p k c"))
                        nc.scalar.dma_start(
                            out=wqkv[:, :, 512:1024],
                            in_=qkvw_d[:, :, 2048 + h * HDV: 2048 + (h + 1) * HDV].rearrange("k p c -> p k c"))

                        qsf = wrk.tile([P, 2, L], bf16, tag="qsf")
                        qsb = wrk.tile([P, 2, L], bf16, tag="qsb")
                        ksf = wrk.tile([P, 2, L], bf16, tag="ksf")
                        ksb = wrk.tile([P, 2, L], bf16, tag="ksb")
                        for ct in range(2):
                            # decays: fwd cs in t1; bwd reverse-inclusive cs in tl
                            t1 = wrk.tile([P, L], f32, tag="t1")
                            t2 = wrk.tile([P, L], f32, tag="t2")
                            tl = wrk.tile([P, L], f32, tag="tl")
                            nc.scalar.activation(tl, sgf[:, hl * 4 + ct, :], AF.Ln)
                            nc.vector.tensor_tensor_scan(t1, tl, zeros, 0.0,
                                                         ALU.add, ALU.add)
                            nc.scalar.activation(tl, sgf[:, hl * 4 + 2 + ct, :], AF.Ln)
                            nc.vector.tensor_tensor_scan(t2, tl, zeros, 0.0,
                                                         ALU.add, ALU.add)
                            # reverse-inclusive cumsum: ls - cs + total  (into tl)
                            nc.vector.tensor_sub(tl, tl, t2)
                            nc.vector.tensor_scalar_add(tl, tl, t2[:, L - 1: L])
                            eqf = wrk.tile([P, L], bf16, tag="eqf")
                            ekf = wrk.tile([P, L], bf16, tag="ekf")
                            eqb = wrk.tile([P, L], bf16, tag="eqb")
                            ekb = wrk.tile([P, L], bf16, tag="ekb")
                            nc.scalar.activation(eqf, t1, AF.Exp, scale=1.0 / GLN)
                            nc.scalar.activation(ekf, t1, AF.Exp, scale=-1.0 / GLN)
                            nc.scalar.activation(eqb, tl, AF.Exp, scale=1.0 / GLN)
                            nc.scalar.activation(ekb, tl, AF.Exp, scale=-1.0 / GLN)
                            for tc2 in range(2):
                                o0, w0 = TC2[tc2]
                                sl = slice(o0, o0 + w0)
                                pt = psum(P, w0)
                                for kt in range(8):
                                    nc.tensor.matmul(pt, wqkv[:, kt, ct * P:(ct + 1) * P],
                                                     xc[:, kt, o0:o0 + w0],
                                                     start=(kt == 0), stop=(kt == 7))
                                nc.vector.tensor_mul(qsf[:, ct, sl], pt, eqf[:, sl])
                                nc.vector.tensor_mul(qsb[:, ct, sl], pt, eqb[:, sl])
                                pt = psum(P, w0)
                                for kt in range(8):
                                    nc.tensor.matmul(pt, wqkv[:, kt, 256 + ct * P: 256 + (ct + 1) * P],
                                                     xc[:, kt, o0:o0 + w0],
                                                     start=(kt == 0), stop=(kt == 7))
                                nc.vector.tensor_mul(ksf[:, ct, sl], pt, ekf[:, sl])
                                nc.vector.tensor_mul(ksb[:, ct, sl], pt, ekb[:, sl])

                        # v projection (token-major)
                        vh = wrk.tile([P, NT7, HDV], bf16, tag="vh")
                        for tt in range(NT7):
                            tw = TW[tt]
                            pt = psum(tw, HDV)
                            for kt in range(8):
                                nc.tensor.matmul(pt, xc[:, kt, tt * P: tt * P + tw],
                                                 wqkv[:, kt, 512:1024],
                                                 start=(kt == 0), stop=(kt == 7))
                            nc.scalar.copy(vh[:tw, tt, :], pt)

                        # A + o per direction
                        for dr in range(2):
                            qs = qsf if dr == 0 else qsb
                            ks = ksf if dr == 0 else ksb
                            am = wrk.tile([P, NT7, L], bf16, tag="am")
                            for j in range(2):
                                jo, jw = ACH[j]
                                for si in range(NT7):
                                    d = si - 4 * j
                                    if dr == 0:
                                        if si * P > jo + jw - 1:
                                            continue        # fully masked
                                        mi_ = None if d < 0 else d
                                    else:
                                        if si * P + SW[si] - 1 < jo:
                                            continue
                                        mi_ = None if d >= 4 else 4 + d
                                    sw = SW[si]
                                    pt = psum(sw, jw)
                                    for ct in range(2):
                                        nc.tensor.matmul(pt, ks[:, ct, si * P: si * P + sw],
                                                         qs[:, ct, jo: jo + jw],
                                                         start=(ct == 0), stop=(ct == 1))
                                    if mi_ is None:
                                        nc.vector.tensor_copy(am[:sw, si, jo: jo + jw], pt)
                                    else:
                                        nc.vector.tensor_mul(am[:sw, si, jo: jo + jw], pt,
                                                             masks[:sw, mi_, :jw])

                            ssq = wrk.tile([P, 8], f32, tag="ssq", bufs=2)
                            nc.vector.memset(ssq[:], 0.0)
                            scrap = wrk.tile([P, HDV], bf16, tag="scrap")
                            for tt in range(NT7):
                                tw = TW[tt]
                                sis = list(range(0, tt + 1) if dr == 0 else range(tt, NT7))
                                pt = psum(tw, HDV)
                                for ii, si in enumerate(sis):
                                    nc.tensor.matmul(pt, am[:SW[si], si, tt * P: tt * P + tw],
                                                     vh[:SW[si], si, :],
                                                     start=(ii == 0), stop=(ii == len(sis) - 1))
                                nc.scalar.activation(scrap[:tw], pt, AF.Square,
                                                     accum_out=ssq[:tw, tt: tt + 1])
                                # rsl = (ssq/512 + eps)^-1/2 = exp(-0.5*ln(...))
                                rsl = wrk.tile([P, 1], f32, tag="rsl", bufs=2)
                                nc.scalar.activation(rsl[:tw], ssq[:tw, tt: tt + 1],
                                                     AF.Ln, scale=1.0 / HDV, bias=epst[:tw])
                                nc.scalar.activation(rsl[:tw], rsl[:tw], AF.Exp, scale=-0.5)
                                oslc = og[:tw, tt, h * HDV:(h + 1) * HDV]
                                if dr == 0:
                                    nc.vector.tensor_scalar_mul(oslc, pt, rsl[:tw])
                                else:
                                    nc.vector.scalar_tensor_tensor(
                                        oslc, pt, rsl[:tw], oslc, ALU.mult, ALU.add)

                # ============ stage E: out = (ogT * gateF)^T-proj ============
                for nch in range(2):
                    owS = wst.tile([P, 16, 512], bf16, tag="wqkv", bufs=2, name="owS")
                    nc.sync.dma_start(
                        out=owS,
                        in_=ow_d[:, :, nch * 512:(nch + 1) * 512].rearrange("j p c -> p j c"))
                    for tt in range(NT7):
                        tw = TW[tt]
                        ogT = est.tile([P, 16, P], bf16, tag="ogT")
                        for g in range(4):
                            ptT = ps.tile([P, 4, P], bf16, tag="psT", bufs=2, name="ptT")
                            for i in range(4):
                                jt = g * 4 + i
                                nc.tensor.transpose(ptT[:, i, :tw],
                                                    og[:tw, tt, jt * P:(jt + 1) * P],
                                                    ident[:tw, :tw])
                            nc.vector.tensor_mul(
                                ogT[:, g * 4:(g + 1) * 4, :tw],
                                ptT[:, :, :tw],
                                gateF[:, g * 4:(g + 1) * 4, tt * P: tt * P + tw])
                        pt = psum(tw, 512)
                        for jt in range(16):
                            nc.tensor.matmul(pt, ogT[:, jt, :tw], owS[:, jt, :],
                                             start=(jt == 0), stop=(jt == 15))
                        outs = sm.tile([P, 512], f32, tag="outs", bufs=2)
                        nc.vector.tensor_copy(outs[:tw], pt)
                        nc.sync.dma_start(
                            out=out_d[bi * L + tt * P: bi * L + tt * P + tw,
                                      nch * 512:(nch + 1) * 512],
                            in_=outs[:tw, :])

    _legalize_sync_waits(nc)
    return nc


_CACHE = {}


def _prep_shared(conv_w, qkv_w, gk_w1, gk_w2, gk_b2, g_w, o_w, gnorm_w, lnorm_w):
    bf = ml_dtypes.bfloat16
    cdg = np.zeros((9, 8, P, P), np.float32)
    w9 = conv_w.reshape(9, D)  # taps x channels (HWIO with I=1)
    idx = np.arange(P)
    for tap in range(9):
        for ft in range(8):
            cdg[tap, ft, idx, idx] = w9[tap, ft * P:(ft + 1) * P]
    assert np.allclose(gnorm_w, lnorm_w), "kernel assumes gnorm_w == lnorm_w (fold into o_w)"
    ow_eff = o_w * np.tile(gnorm_w, NH)[:, None]
    masks = np.zeros((8, P, 512), np.float32)
    s_i = np.arange(P)[:, None]
    t_i = np.arange(512)[None, :]
    for dd in range(4):
        masks[dd] = (s_i <= t_i - P * dd)
        masks[4 + dd] = (s_i >= t_i - P * dd)
    return {
        "cdg": np.ascontiguousarray(cdg.astype(bf)),
        "qkvw": np.ascontiguousarray(qkv_w.reshape(8, P, 4096).astype(bf)),
        "gk1w": np.ascontiguousarray(gk_w1.reshape(8, P, 16).astype(bf)),
        "gk2w": np.ascontiguousarray(gk_w2.astype(bf)),
        "b2": np.ascontiguousarray(gk_b2.reshape(16, P, 1).astype(np.float32)),
        "gw": np.ascontiguousarray(g_w.reshape(8, P, 2048).astype(bf)),
        "ow": np.ascontiguousarray(ow_eff.reshape(16, P, 1024).astype(bf)),
        "masks": np.ascontiguousarray(masks.astype(bf)),
        "ident": np.ascontiguousarray(np.eye(P, dtype=np.float32).astype(bf)),
    }


def kernel(x, conv_w, qkv_w, gk_w1, gk_w2, gk_b2, g_w, g_b, o_w, gnorm_w, lnorm_w, H, W,
           _return_res=False, _trace=False):
    x = np.asarray(x, np.float32)
    assert int(H) == 28 and int(W) == 28 and x.shape == (16, L, D)
    assert np.allclose(np.asarray(g_b), 0.0), "kernel assumes g_b == 0"
    bf = ml_dtypes.bfloat16

    if "nc" not in _CACHE:
        _CACHE["nc"] = _build_program()
    nc = _CACHE["nc"]

    shared = _prep_shared(np.asarray(conv_w, np.float32), np.asarray(qkv_w, np.float32),
                          np.asarray(gk_w1, np.float32), np.asarray(gk_w2, np.float32),
                          np.asarray(gk_b2, np.float32), np.asarray(g_w, np.float32),
                          np.asarray(o_w, np.float32), np.asarray(gnorm_w, np.float32),
                          np.asarray(lnorm_w, np.float32))
    in_maps = []
    for c in range(NCORES):
        xs = x[2 * c: 2 * c + 2]                       # [2, 784, 1024]
        xt = xs.reshape(B, 28, 28, D).transpose(3, 0, 1, 2)   # [1024, 2, 28, 28]
        xpad = np.zeros((D, B, 30, 30), np.float32)
        xpad[:, :, 1:29, 1:29] = xt
        m = dict(shared)
        m["xpad"] = np.ascontiguousarray(xpad.reshape(8, P, B * 900).astype(bf))
        in_maps.append(m)

    res = run_bass_kernel_spmd(nc, in_maps, core_ids=list(range(NCORES)), trace=_trace)
    out = np.concatenate([r["out"].reshape(B, L, D) for r in res.results], axis=0)
    if _return_res:
        return out, res
    return out


# revision 11
# speedup vs baseline: 1.7242x; 1.2007x over previous
"""Trainium2 Bass kernel for nn_GatedLinearAttention (bidirectional GLA vision block).

Strategy (v2)
-------------
Data-parallel over batch: 16 batch items -> 8 cores x 2 items. No collectives.

The chunked GLA scan is reformulated as quadratic causal attention with global
decay (exact):  o_t = sum_{s<=t} exp(B_t - B_s) (q_t . k_s) v_s, B = cumsum of
log-gates.  qs = q*exp(B), ks = k*exp(-B); backward direction = anti-causal
mask with reverse cumsum.

v2 changes vs v1:
 - Stage E output projection uses PE (identity-matmul) transposes instead of
   DMA transposes (v1 spent ~556us on a serialized DMA-transpose queue).
 - Activation-table thrash removed: per batch item the schedule is grouped
   into [sigmoid-set phases] (conv/gate/decay sigmoids) and [ln-exp-set
   phases] (everything else; Copy/Square are in every set).
 - Gate is computed feature-major and fused into the stage-E transpose copy
   (one DVE mul per tile) instead of a separate token-major gate pass.
 - RMS scaling is fused into the PSUM->SBUF write of the attention output
   (per-token-tile rsqrt computed on the spot).
 - PSUM->SBUF copies moved off the scalar engine (GpSimd/Vector).
 - Weight/descriptor DMA split across the two HWDGE queues (sync + scalar).
"""

import os
import sys
from contextlib import ExitStack

for _p in ("/opt/trn_rl_repo", "/root/.axon_site/_ro/trn_rl_repo"):
    if os.path.isdir(_p) and _p not in sys.path:
        sys.path.insert(0, _p)

import numpy as np
import ml_dtypes

import concourse.bass as bass
import concourse.tile as tile
import concourse.mybir as mybir
from concourse.bass_utils import run_bass_kernel_spmd

f32 = mybir.dt.float32
bf16 = mybir.dt.bfloat16
AF = mybir.ActivationFunctionType
ALU = mybir.AluOpType

P = 128
NCORES = 8
B = 2               # batch items per core
L = 784             # tokens per batch item (28*28)
T = B * L           # tokens per core
D = 1024            # d_model
NH = 4
HDK = 256           # per-head key dim (2 partition tiles)
HDV = 512           # per-head value dim
GLN = 16.0
EPS = 1e-5
NT7 = 7             # token tiles per batch item (6*128 + 16)
TW = [128, 128, 128, 128, 128, 128, 16]
SW = TW
TC2 = [(0, 392), (392, 392)]              # 392-col token chunks
ACH = [(0, 512), (512, 272)]              # A-phase t-chunks


def _legalize_sync_waits(nc, max_waits=1):
    """The walrus shipped here rejects >1 semaphore wait per instruction.
    Split excess waits onto chained NOPs on the same engine right before the
    offending instruction: engines run their stream in order, so blocking
    earlier is equivalent."""
    counter = 0
    for fn in nc.m.functions:
        for blk in fn.blocks:
            insts = list(blk.instructions)
            changed = False
            out = []
            for inst in insts:
                si = inst.sync_info
                if si is not None and len(si.on_wait) > max_waits:
                    waits = list(si.on_wait)
                    keep = waits[len(waits) - max_waits:]
                    move = waits[: len(waits) - max_waits]
                    for i in range(0, len(move), max_waits):
                        chunk = move[i: i + max_waits]
                        nop = mybir.InstNoOp(
                            name=f"legalize-wait-nop-{counter}", ins=[], outs=[]
                        )
                        counter += 1
                        nop.engine = inst.engine
                        nop.sync_info = mybir.SyncInfo(on_wait=chunk, on_update=[])
                        out.append(nop)
                    inst.sync_info = mybir.SyncInfo(
                        on_wait=keep, on_update=list(si.on_update)
                    )
                    changed = True
                out.append(inst)
            if changed:
                blk.instructions = out


def _build_program():
    nc = bass.Bass()

    xpad_d = nc.dram_tensor("xpad", [8, P, B * 30 * 30], bf16, kind="ExternalInput")
    cdg_d = nc.dram_tensor("cdg", [9, 8, P, P], bf16, kind="ExternalInput")
    qkvw_d = nc.dram_tensor("qkvw", [8, P, 4096], bf16, kind="ExternalInput")
    gk1w_d = nc.dram_tensor("gk1w", [8, P, 16], bf16, kind="ExternalInput")
    gk2w_d = nc.dram_tensor("gk2w", [16, 2048], bf16, kind="ExternalInput")
    b2_d = nc.dram_tensor("b2", [16, P, 1], f32, kind="ExternalInput")
    gw_d = nc.dram_tensor("gw", [8, P, 2048], bf16, kind="ExternalInput")
    ow_d = nc.dram_tensor("ow", [16, P, 1024], bf16, kind="ExternalInput")
    masks_d = nc.dram_tensor("masks", [8, P, 512], bf16, kind="ExternalInput")
    ident_d = nc.dram_tensor("ident", [P, P], bf16, kind="ExternalInput")
    out_d = nc.dram_tensor("out", [T, 1024], f32, kind="ExternalOutput")

    with tile.TileContext(nc) as tc:
        with ExitStack() as ctx:
            cst = ctx.enter_context(tc.tile_pool(name="cst", bufs=1))
            big = ctx.enter_context(tc.tile_pool(name="big", bufs=1))
            wst = ctx.enter_context(tc.tile_pool(name="wst", bufs=1))
            wrk = ctx.enter_context(tc.tile_pool(name="wrk", bufs=1))
            sm = ctx.enter_context(tc.tile_pool(name="sm", bufs=1))
            est = ctx.enter_context(tc.tile_pool(name="est", bufs=1))
            ps = ctx.enter_context(tc.tile_pool(name="ps", bufs=1, space="PSUM"))

            def psum(rows, cols):
                pstile = ps.tile([P, 512], f32, tag="ps", bufs=6, name="pstile")
                return pstile[:rows, :cols]

            # ---- constants ----
            masks = cst.tile([P, 8, 512], bf16)
            nc.sync.dma_start(out=masks, in_=masks_d.rearrange("m p t -> p m t"))
            ident = cst.tile([P, P], bf16)
            nc.sync.dma_start(out=ident, in_=ident_d[:, :])
            zeros = cst.tile([P, L], bf16)
            nc.vector.memset(zeros[:], 0.0)
            epst = cst.tile([P, 1], f32)
            nc.vector.memset(epst[:], EPS)
            zeros512 = cst.tile([P, 512], bf16)
            nc.vector.memset(zeros512[:], 0.0)
            w1 = cst.tile([P, 8, 16], bf16)
            nc.sync.dma_start(out=w1, in_=gk1w_d.rearrange("k p c -> p k c"))
            w2 = cst.tile([16, 16, P], bf16)
            nc.sync.dma_start(out=w2, in_=gk2w_d.rearrange("k (m p) -> k m p", m=16))
            b2t = cst.tile([P, 16], f32)
            nc.sync.dma_start(out=b2t, in_=b2_d.rearrange("m p o -> p (m o)"))

            for bi in range(B):
                # persistent per-bi slabs (tag reuse across bi)
                xc = big.tile([P, 8, L], bf16, tag="xc")
                gk1o = big.tile([16, L], bf16, tag="gk1o")
                gateF = big.tile([P, 16, L], bf16, tag="gateF")
                og = big.tile([P, NT7, 2048], bf16, tag="og")
                sgf = big.tile([P, 8, L], f32, tag="sgf")   # decay sigmoids for 2 heads

                # ============ sigma-phase A: conv, gk1, gateF, decay(h0,h1) ============
                # conv 3x3 depthwise + silu (sigmoid table)
                for ft in range(8):
                    xp = wst.tile([P, 30, 30], bf16, tag="xp", bufs=2)
                    nc.sync.dma_start(
                        out=xp,
                        in_=xpad_d[ft, :, bi * 900:(bi + 1) * 900].rearrange(
                            "p (h w) -> p h w", h=30))
                    cd = wst.tile([P, 9, P], bf16, tag="cd", bufs=2)
                    nc.sync.dma_start(out=cd, in_=cdg_d[:, ft].rearrange("m p q -> p m q"))
                    for half in range(2):
                        pt = psum(P, 392)
                        for tap in range(9):
                            a, bb = tap // 3, tap % 3
                            rhs = xp[:, a + half * 14: a + half * 14 + 14, bb: bb + 28]
                            nc.tensor.matmul(pt, cd[:, tap, :], rhs,
                                             start=(tap == 0), stop=(tap == 8))
                        sgc = sm.tile([P, 392], bf16, tag="sgc", bufs=2)
                        nc.scalar.activation(sgc, pt, AF.Sigmoid)
                        nc.vector.tensor_mul(xc[:, ft, half * 392:(half + 1) * 392], pt, sgc)

                # gk1 bottleneck [16, L]
                for tc2 in range(2):
                    o0, w0 = TC2[tc2]
                    pt = psum(16, w0)
                    for kt in range(8):
                        nc.tensor.matmul(pt, w1[:, kt, :], xc[:, kt, o0:o0 + w0],
                                         start=(kt == 0), stop=(kt == 7))
                    nc.scalar.copy(gk1o[:, o0:o0 + w0], pt)

                # gate, feature-major: gateF[jt, t] = silu(gw^T xc)
                for jt in range(16):
                    gwj = wst.tile([P, 8, P], bf16, tag="gwj", bufs=2)
                    nc.scalar.dma_start(
                        out=gwj,
                        in_=gw_d[:, :, jt * P:(jt + 1) * P].rearrange("k p c -> p k c"))
                    for tc2 in range(2):
                        o0, w0 = TC2[tc2]
                        pt = psum(P, w0)
                        for kt in range(8):
                            nc.tensor.matmul(pt, gwj[:, kt, :], xc[:, kt, o0:o0 + w0],
                                             start=(kt == 0), stop=(kt == 7))
                        sgc = sm.tile([P, 392], bf16, tag="sgc", bufs=2)
                        nc.scalar.activation(sgc[:, :w0], pt, AF.Sigmoid)
                        nc.vector.tensor_mul(gateF[:, jt, o0:o0 + w0], pt, sgc[:, :w0])

                for hh in range(2):           # two half-rounds: heads (0,1) then (2,3)
                    # ---- decay-u sigmoids for this pair of heads ----
                    if hh == 1:
                        sgf = big.tile([P, 8, L], f32, tag="sgf")
                    for hl in range(2):
                        h = hh * 2 + hl
                        for dr in range(2):
                            for ct in range(2):
                                mi_g = dr * 8 + h * 2 + ct
                                slot = hl * 4 + dr * 2 + ct
                                for tc2 in range(2):
                                    o0, w0 = TC2[tc2]
                                    pt = psum(P, w0)
                                    nc.tensor.matmul(pt, w2[:, mi_g, :],
                                                     gk1o[:, o0:o0 + w0],
                                                     start=True, stop=True)
                                    nc.scalar.activation(
                                        sgf[:, slot, o0:o0 + w0], pt, AF.Sigmoid,
                                        bias=b2t[:, mi_g: mi_g + 1])

                    # ---- ln/exp phase: attention for heads hh*2, hh*2+1 ----
                    for hl in range(2):
                        h = hh * 2 + hl
                        wqkv = wst.tile([P, 8, 1024], bf16, tag="wqkv", bufs=2)
                        nc.sync.dma_start(
                            out=wqkv[:, :, 0:256],
                            in_=qkvw_d[:, :, h * HDK:(h + 1) * HDK].rearrange("k p c -> p k c"))
                        nc.sync.dma_start(
                            out=wqkv[:, :, 256:512],
                            in_=qkvw_d[:, :, 1024 + h * HDK: 1024 + (h + 1) * HDK].rearrange("k p c -> p k c"))
                        nc.sync.dma_start(
                            out=wqkv[:, :, 512:1024],
                            in_=qkvw_d[:, :, 2048 + h * HDV: 2048 + (h + 1) * HDV].rearrange("k p c -> p k c"))

                        qsf = wrk.tile([P, 2, L], bf16, tag="qsf")
                        qsb = wrk.tile([P, 2, L], bf16, tag="qsb")
                        ksf = wrk.tile([P, 2, L], bf16, tag="ksf")
                        ksb = wrk.tile([P, 2, L], bf16, tag="ksb")
                        for ct in range(2):
                            # decays: fwd cs in t1; bwd reverse-inclusive cs in tl
                            t1 = wrk.tile([P, L], f32, tag="t1")
                            t2 = wrk.tile([P, L], f32, tag="t2")
                            tl = wrk.tile([P, L], f32, tag="tl")
                            nc.scalar.activation(tl, sgf[:, hl * 4 + ct, :], AF.Ln)
                            nc.vector.tensor_tensor_scan(t1, tl, zeros, 0.0,
                                                         ALU.add, ALU.add)
                            nc.scalar.activation(tl, sgf[:, hl * 4 + 2 + ct, :], AF.Ln)
                            nc.vector.tensor_tensor_scan(t2, tl, zeros, 0.0,
                                                         ALU.add, ALU.add)
                            # reverse-inclusive cumsum: ls - cs + total  (into tl)
                            nc.vector.tensor_sub(tl, tl, t2)
                            nc.vector.tensor_scalar_add(tl, tl, t2[:, L - 1: L])
                            eqf = wrk.tile([P, L], bf16, tag="eqf")
                            ekf = wrk.tile([P, L], bf16, tag="ekf")
                            eqb = wrk.tile([P, L], bf16, tag="eqb")
                            ekb = wrk.tile([P, L], bf16, tag="ekb")
                            nc.scalar.activation(eqf, t1, AF.Exp, scale=1.0 / GLN)
                            nc.scalar.activation(ekf, t1, AF.Exp, scale=-1.0 / GLN)
                            nc.scalar.activation(eqb, tl, AF.Exp, scale=1.0 / GLN)
                            nc.scalar.activation(ekb, tl, AF.Exp, scale=-1.0 / GLN)
                            for tc2 in range(2):
                                o0, w0 = TC2[tc2]
                                sl = slice(o0, o0 + w0)
                                pt = psum(P, w0)
                                for kt in range(8):
                                    nc.tensor.matmul(pt, wqkv[:, kt, ct * P:(ct + 1) * P],
                                                     xc[:, kt, o0:o0 + w0],
                                                     start=(kt == 0), stop=(kt == 7))
                                nc.vector.tensor_mul(qsf[:, ct, sl], pt, eqf[:, sl])
                                nc.vector.tensor_mul(qsb[:, ct, sl], pt, eqb[:, sl])
                                pt = psum(P, w0)
                                for kt in range(8):
                                    nc.tensor.matmul(pt, wqkv[:, kt, 256 + ct * P: 256 + (ct + 1) * P],
                                                     xc[:, kt, o0:o0 + w0],
                                                     start=(kt == 0), stop=(kt == 7))
                                nc.vector.tensor_mul(ksf[:, ct, sl], pt, ekf[:, sl])
                                nc.vector.tensor_mul(ksb[:, ct, sl], pt, ekb[:, sl])

                        # v projection (token-major)
                        vh = wrk.tile([P, NT7, HDV], bf16, tag="vh")
                        for tt in range(NT7):
                            tw = TW[tt]
                            pt = psum(tw, HDV)
                            for kt in range(8):
                                nc.tensor.matmul(pt, xc[:, kt, tt * P: tt * P + tw],
                                                 wqkv[:, kt, 512:1024],
                                                 start=(kt == 0), stop=(kt == 7))
                            nc.scalar.copy(vh[:tw, tt, :], pt)

                        # A + o per direction
                        for dr in range(2):
                            qs = qsf if dr == 0 else qsb
                            ks = ksf if dr == 0 else ksb
                            am = wrk.tile([P, NT7, L], bf16, tag="am")
                            for j in range(2):
                                jo, jw = ACH[j]
                                for si in range(NT7):
                                    d = si - 4 * j
                                    if dr == 0:
                                        if si * P > jo + jw - 1:
                                            continue        # fully masked
                                        mi_ = None if d < 0 else d
                                    else:
                                        if si * P + SW[si] - 1 < jo:
                                            continue
                                        mi_ = None if d >= 4 else 4 + d
                                    sw = SW[si]
                                    pt = psum(sw, jw)
                                    for ct in range(2):
                                        nc.tensor.matmul(pt, ks[:, ct, si * P: si * P + sw],
                                                         qs[:, ct, jo: jo + jw],
                                                         start=(ct == 0), stop=(ct == 1))
                                    if mi_ is None:
                                        nc.vector.tensor_copy(am[:sw, si, jo: jo + jw], pt)
                                    else:
                                        nc.vector.tensor_mul(am[:sw, si, jo: jo + jw], pt,
                                                             masks[:sw, mi_, :jw])

                            ssq = wrk.tile([P, 8], f32, tag="ssq", bufs=2)
                            nc.vector.memset(ssq[:], 0.0)
                            scrap = wrk.tile([P, HDV], bf16, tag="scrap")
                            for tt in range(NT7):
                                tw = TW[tt]
                                sis = list(range(0, tt + 1) if dr == 0 else range(tt, NT7))
                                pt = psum(tw, HDV)
                                for ii, si in enumerate(sis):
                                    nc.tensor.matmul(pt, am[:SW[si], si, tt * P: tt * P + tw],
                                                     vh[:SW[si], si, :],
                                                     start=(ii == 0), stop=(ii == len(sis) - 1))
                                nc.scalar.activation(scrap[:tw], pt, AF.Square,
                                                     accum_out=ssq[:tw, tt: tt + 1])
                                # rsl = (ssq/512 + eps)^-1/2 = exp(-0.5*ln(...))
                                rsl = wrk.tile([P, 1], f32, tag="rsl", bufs=2)
                                nc.scalar.activation(rsl[:tw], ssq[:tw, tt: tt + 1],
                                                     AF.Ln, scale=1.0 / HDV, bias=epst[:tw])
                                nc.scalar.activation(rsl[:tw], rsl[:tw], AF.Exp, scale=-0.5)
                                oslc = og[:tw, tt, h * HDV:(h + 1) * HDV]
                                if dr == 0:
                                    nc.vector.scalar_tensor_tensor(
                                        oslc, pt, rsl[:tw], zeros512[:tw], ALU.mult, ALU.add)
                                else:
                                    nc.vector.scalar_tensor_tensor(
                                        oslc, pt, rsl[:tw], oslc, ALU.mult, ALU.add)

                # ============ stage E: out = (ogT * gateF) @ ow ============
                owS0 = wst.tile([P, 16, 512], bf16, tag="wqkv", bufs=2, name="owS0")
                nc.sync.dma_start(
                    out=owS0, in_=ow_d[:, :, 0:512].rearrange("j p c -> p j c"))
                owS1 = wst.tile([P, 16, 512], bf16, tag="wqkv", bufs=2, name="owS1")
                nc.sync.dma_start(
                    out=owS1, in_=ow_d[:, :, 512:1024].rearrange("j p c -> p j c"))
                for tt in range(NT7):
                    tw = TW[tt]
                    ogT = est.tile([P, 16, P], bf16, tag="ogT")
                    for g in range(4):
                        ptT = ps.tile([P, 4, P], bf16, tag="psT", bufs=2, name="ptT")
                        for i in range(4):
                            jt = g * 4 + i
                            nc.tensor.transpose(ptT[:, i, :tw],
                                                og[:tw, tt, jt * P:(jt + 1) * P],
                                                ident[:tw, :tw])
                        nc.vector.tensor_mul(
                            ogT[:, g * 4:(g + 1) * 4, :tw],
                            ptT[:, :, :tw],
                            gateF[:, g * 4:(g + 1) * 4, tt * P: tt * P + tw])
                    for nch, owS in ((0, owS0), (1, owS1)):
                        pt = psum(tw, 512)
                        for jt in range(16):
                            nc.tensor.matmul(pt, ogT[:, jt, :tw], owS[:, jt, :],
                                             start=(jt == 0), stop=(jt == 15))
                        outs = sm.tile([P, 512], f32, tag="outs", bufs=2)
                        nc.vector.tensor_copy(outs[:tw], pt)
                        nc.sync.dma_start(
                            out=out_d[bi * L + tt * P: bi * L + tt * P + tw,
                                      nch * 512:(nch + 1) * 512],
                            in_=outs[:tw, :])

    _legalize_sync_waits(nc)
    return nc


_CACHE = {}


def _prep_shared(conv_w, qkv_w, gk_w1, gk_w2, gk_b2, g_w, o_w, gnorm_w, lnorm_w):
    bf = ml_dtypes.bfloat16
    cdg = np.zeros((9, 8, P, P), np.float32)
    w9 = conv_w.reshape(9, D)  # taps x channels (HWIO with I=1)
    idx = np.arange(P)
    for tap in range(9):
        for ft in range(8):
            cdg[tap, ft, idx, idx] = w9[tap, ft * P:(ft + 1) * P]
    assert np.allclose(gnorm_w, lnorm_w), "kernel assumes gnorm_w == lnorm_w (fold into o_w)"
    ow_eff = o_w * np.tile(gnorm_w, NH)[:, None]
    masks = np.zeros((8, P, 512), np.float32)
    s_i = np.arange(P)[:, None]
    t_i = np.arange(512)[None, :]
    for dd in range(4):
        masks[dd] = (s_i <= t_i - P * dd)
        masks[4 + dd] = (s_i >= t_i - P * dd)
    return {
        "cdg": np.ascontiguousarray(cdg.astype(bf)),
        "qkvw": np.ascontiguousarray(qkv_w.reshape(8, P, 4096).astype(bf)),
        "gk1w": np.ascontiguousarray(gk_w1.reshape(8, P, 16).astype(bf)),
        "gk2w": np.ascontiguousarray(gk_w2.astype(bf)),
        "b2": np.ascontiguousarray(gk_b2.reshape(16, P, 1).astype(np.float32)),
        "gw": np.ascontiguousarray(g_w.reshape(8, P, 2048).astype(bf)),
        "ow": np.ascontiguousarray(ow_eff.reshape(16, P, 1024).astype(bf)),
        "masks": np.ascontiguousarray(masks.astype(bf)),
        "ident": np.ascontiguousarray(np.eye(P, dtype=np.float32).astype(bf)),
    }


def kernel(x, conv_w, qkv_w, gk_w1, gk_w2, gk_b2, g_w, g_b, o_w, gnorm_w, lnorm_w, H, W,
           _return_res=False, _trace=False):
    x = np.asarray(x, np.float32)
    assert int(H) == 28 and int(W) == 28 and x.shape == (16, L, D)
    assert np.allclose(np.asarray(g_b), 0.0), "kernel assumes g_b == 0"
    bf = ml_dtypes.bfloat16

    if "nc" not in _CACHE:
        _CACHE["nc"] = _build_program()
    nc = _CACHE["nc"]

    shared = _prep_shared(np.asarray(conv_w, np.float32), np.asarray(qkv_w, np.float32),
                          np.asarray(gk_w1, np.float32), np.asarray(gk_w2, np.float32),
                          np.asarray(gk_b2, np.float32), np.asarray(g_w, np.float32),
                          np.asarray(o_w, np.float32), np.asarray(gnorm_w, np.float32),
                          np.asarray(lnorm_w, np.float32))
    in_maps = []
    for c in range(NCORES):
        xs = x[2 * c: 2 * c + 2]                       # [2, 784, 1024]
        xt = xs.reshape(B, 28, 28, D).transpose(3, 0, 1, 2)   # [1024, 2, 28, 28]
        xpad = np.zeros((D, B, 30, 30), np.float32)
        xpad[:, :, 1:29, 1:29] = xt
        m = dict(shared)
        m["xpad"] = np.ascontiguousarray(xpad.reshape(8, P, B * 900).astype(bf))
        in_maps.append(m)

    res = run_bass_kernel_spmd(nc, in_maps, core_ids=list(range(NCORES)), trace=_trace)
    out = np.concatenate([r["out"].reshape(B, L, D) for r in res.results], axis=0)
    if _return_res:
        return out, res
    return out


# revision 12
# speedup vs baseline: 1.7926x; 1.0397x over previous
"""Trainium2 Bass kernel for nn_GatedLinearAttention (bidirectional GLA vision block).

Strategy (v2)
-------------
Data-parallel over batch: 16 batch items -> 8 cores x 2 items. No collectives.

The chunked GLA scan is reformulated as quadratic causal attention with global
decay (exact):  o_t = sum_{s<=t} exp(B_t - B_s) (q_t . k_s) v_s, B = cumsum of
log-gates.  qs = q*exp(B), ks = k*exp(-B); backward direction = anti-causal
mask with reverse cumsum.

v2 changes vs v1:
 - Stage E output projection uses PE (identity-matmul) transposes instead of
   DMA transposes (v1 spent ~556us on a serialized DMA-transpose queue).
 - Activation-table thrash removed: per batch item the schedule is grouped
   into [sigmoid-set phases] (conv/gate/decay sigmoids) and [ln-exp-set
   phases] (everything else; Copy/Square are in every set).
 - Gate is computed feature-major and fused into the stage-E transpose copy
   (one DVE mul per tile) instead of a separate token-major gate pass.
 - RMS scaling is fused into the PSUM->SBUF write of the attention output
   (per-token-tile rsqrt computed on the spot).
 - PSUM->SBUF copies moved off the scalar engine (GpSimd/Vector).
 - Weight/descriptor DMA split across the two HWDGE queues (sync + scalar).
"""

import os
import sys
from contextlib import ExitStack

for _p in ("/opt/trn_rl_repo", "/root/.axon_site/_ro/trn_rl_repo"):
    if os.path.isdir(_p) and _p not in sys.path:
        sys.path.insert(0, _p)

import numpy as np
import ml_dtypes

import concourse.bass as bass
import concourse.tile as tile
import concourse.mybir as mybir
from concourse.bass_utils import run_bass_kernel_spmd

f32 = mybir.dt.float32
bf16 = mybir.dt.bfloat16
AF = mybir.ActivationFunctionType
ALU = mybir.AluOpType

P = 128
NCORES = 8
B = 2               # batch items per core
L = 784             # tokens per batch item (28*28)
T = B * L           # tokens per core
D = 1024            # d_model
NH = 4
HDK = 256           # per-head key dim (2 partition tiles)
HDV = 512           # per-head value dim
GLN = 16.0
EPS = 1e-5
NT7 = 7             # token tiles per batch item (6*128 + 16)
TW = [128, 128, 128, 128, 128, 128, 16]
SW = TW
TC2 = [(0, 392), (392, 392)]              # 392-col token chunks
ACH = [(0, 512), (512, 272)]              # A-phase t-chunks


def _legalize_sync_waits(nc, max_waits=1):
    """The walrus shipped here rejects >1 semaphore wait per instruction.
    Split excess waits onto chained NOPs on the same engine right before the
    offending instruction: engines run their stream in order, so blocking
    earlier is equivalent."""
    counter = 0
    for fn in nc.m.functions:
        for blk in fn.blocks:
            insts = list(blk.instructions)
            changed = False
            out = []
            for inst in insts:
                si = inst.sync_info
                if si is not None and len(si.on_wait) > max_waits:
                    waits = list(si.on_wait)
                    keep = waits[len(waits) - max_waits:]
                    move = waits[: len(waits) - max_waits]
                    for i in range(0, len(move), max_waits):
                        chunk = move[i: i + max_waits]
                        nop = mybir.InstNoOp(
                            name=f"legalize-wait-nop-{counter}", ins=[], outs=[]
                        )
                        counter += 1
                        nop.engine = inst.engine
                        nop.sync_info = mybir.SyncInfo(on_wait=chunk, on_update=[])
                        out.append(nop)
                    inst.sync_info = mybir.SyncInfo(
                        on_wait=keep, on_update=list(si.on_update)
                    )
                    changed = True
                out.append(inst)
            if changed:
                blk.instructions = out


def _build_program():
    nc = bass.Bass()

    xpad_d = nc.dram_tensor("xpad", [8, P, B * 30 * 30], bf16, kind="ExternalInput")
    cdg_d = nc.dram_tensor("cdg", [9, 8, P, P], bf16, kind="ExternalInput")
    qkvw_d = nc.dram_tensor("qkvw", [8, P, 4096], bf16, kind="ExternalInput")
    gk1w_d = nc.dram_tensor("gk1w", [8, P, 16], bf16, kind="ExternalInput")
    gk2w_d = nc.dram_tensor("gk2w", [16, 2048], bf16, kind="ExternalInput")
    b2_d = nc.dram_tensor("b2", [16, P, 1], f32, kind="ExternalInput")
    gw_d = nc.dram_tensor("gw", [8, P, 2048], bf16, kind="ExternalInput")
    ow_d = nc.dram_tensor("ow", [16, P, 1024], bf16, kind="ExternalInput")
    masks_d = nc.dram_tensor("masks", [8, P, 512], bf16, kind="ExternalInput")
    ident_d = nc.dram_tensor("ident", [P, P], bf16, kind="ExternalInput")
    out_d = nc.dram_tensor("out", [T, 1024], f32, kind="ExternalOutput")

    with tile.TileContext(nc) as tc:
        with ExitStack() as ctx:
            cst = ctx.enter_context(tc.tile_pool(name="cst", bufs=1))
            big = ctx.enter_context(tc.tile_pool(name="big", bufs=1))
            wst = ctx.enter_context(tc.tile_pool(name="wst", bufs=1))
            wrk = ctx.enter_context(tc.tile_pool(name="wrk", bufs=1))
            sm = ctx.enter_context(tc.tile_pool(name="sm", bufs=1))
            est = ctx.enter_context(tc.tile_pool(name="est", bufs=1))
            ps = ctx.enter_context(tc.tile_pool(name="ps", bufs=1, space="PSUM"))

            def psum(rows, cols):
                pstile = ps.tile([P, 512], f32, tag="ps", bufs=6, name="pstile")
                return pstile[:rows, :cols]

            # ---- constants ----
            masks = cst.tile([P, 8, 512], bf16)
            nc.sync.dma_start(out=masks, in_=masks_d.rearrange("m p t -> p m t"))
            ident = cst.tile([P, P], bf16)
            nc.sync.dma_start(out=ident, in_=ident_d[:, :])
            zeros = cst.tile([P, L], bf16)
            nc.vector.memset(zeros[:], 0.0)
            epst = cst.tile([P, 1], f32)
            nc.vector.memset(epst[:], EPS)
            zeros512 = cst.tile([P, 512], bf16)
            nc.vector.memset(zeros512[:], 0.0)
            w1 = cst.tile([P, 8, 16], bf16)
            nc.sync.dma_start(out=w1, in_=gk1w_d.rearrange("k p c -> p k c"))
            w2 = cst.tile([16, 16, P], bf16)
            nc.sync.dma_start(out=w2, in_=gk2w_d.rearrange("k (m p) -> k m p", m=16))
            b2t = cst.tile([P, 16], f32)
            nc.sync.dma_start(out=b2t, in_=b2_d.rearrange("m p o -> p (m o)"))

            for bi in range(B):
                # persistent per-bi slabs (tag reuse across bi)
                xc = big.tile([P, 8, L], bf16, tag="xc")
                gk1o = big.tile([16, L], bf16, tag="gk1o")
                gateF = big.tile([P, 16, L], bf16, tag="gateF")
                og = big.tile([P, NT7, 2048], bf16, tag="og")
                sgf = big.tile([P, 16, L], bf16, tag="sgf")  # decay sigmoids, all heads

                # ============ sigma-phase A: conv, gk1, gateF, decay(h0,h1) ============
                # conv 3x3 depthwise + silu (sigmoid table)
                for ft in range(8):
                    xp = wst.tile([P, 30, 30], bf16, tag="xp", bufs=2)
                    nc.scalar.dma_start(
                        out=xp,
                        in_=xpad_d[ft, :, bi * 900:(bi + 1) * 900].rearrange(
                            "p (h w) -> p h w", h=30))
                    cd = wst.tile([P, 9, P], bf16, tag="cd", bufs=2)
                    nc.scalar.dma_start(out=cd, in_=cdg_d[:, ft].rearrange("m p q -> p m q"))
                    for half in range(2):
                        pt = psum(P, 392)
                        for tap in range(9):
                            a, bb = tap // 3, tap % 3
                            rhs = xp[:, a + half * 14: a + half * 14 + 14, bb: bb + 28]
                            nc.tensor.matmul(pt, cd[:, tap, :], rhs,
                                             start=(tap == 0), stop=(tap == 8))
                        sgc = sm.tile([P, 392], bf16, tag="sgc", bufs=2)
                        nc.scalar.activation(sgc, pt, AF.Sigmoid)
                        nc.vector.tensor_mul(xc[:, ft, half * 392:(half + 1) * 392], pt, sgc)

                # gk1 bottleneck [16, L]
                for tc2 in range(2):
                    o0, w0 = TC2[tc2]
                    pt = psum(16, w0)
                    for kt in range(8):
                        nc.tensor.matmul(pt, w1[:, kt, :], xc[:, kt, o0:o0 + w0],
                                         start=(kt == 0), stop=(kt == 7))
                    nc.scalar.copy(gk1o[:, o0:o0 + w0], pt)

                # gate, feature-major: gateF[jt, t] = silu(gw^T xc)
                for jt in range(16):
                    gwj = wst.tile([P, 8, P], bf16, tag="gwj", bufs=2)
                    nc.scalar.dma_start(
                        out=gwj,
                        in_=gw_d[:, :, jt * P:(jt + 1) * P].rearrange("k p c -> p k c"))
                    for tc2 in range(2):
                        o0, w0 = TC2[tc2]
                        pt = psum(P, w0)
                        for kt in range(8):
                            nc.tensor.matmul(pt, gwj[:, kt, :], xc[:, kt, o0:o0 + w0],
                                             start=(kt == 0), stop=(kt == 7))
                        sgc = sm.tile([P, 392], bf16, tag="sgc", bufs=2)
                        nc.scalar.activation(sgc[:, :w0], pt, AF.Sigmoid)
                        nc.vector.tensor_mul(gateF[:, jt, o0:o0 + w0], pt, sgc[:, :w0])

                # ---- decay-u sigmoids for all heads (sigmoid table phase) ----
                for h in range(4):
                    for dr in range(2):
                        for ct in range(2):
                            mi_g = dr * 8 + h * 2 + ct
                            slot = h * 4 + dr * 2 + ct
                            for tc2 in range(2):
                                o0, w0 = TC2[tc2]
                                pt = psum(P, w0)
                                nc.tensor.matmul(pt, w2[:, mi_g, :],
                                                 gk1o[:, o0:o0 + w0],
                                                 start=True, stop=True)
                                nc.scalar.activation(
                                    sgf[:, slot, o0:o0 + w0], pt, AF.Sigmoid,
                                    bias=b2t[:, mi_g: mi_g + 1])

                # ---- ln/exp phase: attention for all heads ----
                if True:
                    for h in range(4):
                        wqkv = wst.tile([P, 8, 1024], bf16, tag="wqkv", bufs=2)
                        nc.sync.dma_start(
                            out=wqkv[:, :, 0:256],
                            in_=qkvw_d[:, :, h * HDK:(h + 1) * HDK].rearrange("k p c -> p k c"))
                        nc.sync.dma_start(
                            out=wqkv[:, :, 256:512],
                            in_=qkvw_d[:, :, 1024 + h * HDK: 1024 + (h + 1) * HDK].rearrange("k p c -> p k c"))
                        nc.sync.dma_start(
                            out=wqkv[:, :, 512:1024],
                            in_=qkvw_d[:, :, 2048 + h * HDV: 2048 + (h + 1) * HDV].rearrange("k p c -> p k c"))

                        qsf = wrk.tile([P, 2, L], bf16, tag="qsf")
                        qsb = wrk.tile([P, 2, L], bf16, tag="qsb")
                        ksf = wrk.tile([P, 2, L], bf16, tag="ksf")
                        ksb = wrk.tile([P, 2, L], bf16, tag="ksb")
                        for ct in range(2):
                            # decays: fwd cs in t1; bwd reverse-inclusive cs in tl
                            t1 = wrk.tile([P, L], f32, tag="t1")
                            t2 = wrk.tile([P, L], f32, tag="t2")
                            tl = wrk.tile([P, L], f32, tag="tl")
                            nc.scalar.activation(tl, sgf[:, h * 4 + ct, :], AF.Ln)
                            nc.vector.tensor_tensor_scan(t1, tl, zeros, 0.0,
                                                         ALU.add, ALU.add)
                            nc.scalar.activation(tl, sgf[:, h * 4 + 2 + ct, :], AF.Ln)
                            nc.vector.tensor_tensor_scan(t2, tl, zeros, 0.0,
                                                         ALU.add, ALU.add)
                            # reverse-inclusive cumsum: ls - cs + total  (into tl)
                            nc.vector.tensor_sub(tl, tl, t2)
                            nc.vector.tensor_scalar_add(tl, tl, t2[:, L - 1: L])
                            eqf = wrk.tile([P, L], bf16, tag="eqf")
                            ekf = wrk.tile([P, L], bf16, tag="ekf")
                            eqb = wrk.tile([P, L], bf16, tag="eqb")
                            ekb = wrk.tile([P, L], bf16, tag="ekb")
                            nc.scalar.activation(eqf, t1, AF.Exp, scale=1.0 / GLN)
                            nc.scalar.activation(ekf, t1, AF.Exp, scale=-1.0 / GLN)
                            nc.scalar.activation(eqb, tl, AF.Exp, scale=1.0 / GLN)
                            nc.scalar.activation(ekb, tl, AF.Exp, scale=-1.0 / GLN)
                            for tc2 in range(2):
                                o0, w0 = TC2[tc2]
                                sl = slice(o0, o0 + w0)
                                pt = psum(P, w0)
                                for kt in range(8):
                                    nc.tensor.matmul(pt, wqkv[:, kt, ct * P:(ct + 1) * P],
                                                     xc[:, kt, o0:o0 + w0],
                                                     start=(kt == 0), stop=(kt == 7))
                                nc.vector.tensor_mul(qsf[:, ct, sl], pt, eqf[:, sl])
                                nc.vector.tensor_mul(qsb[:, ct, sl], pt, eqb[:, sl])
                                pt = psum(P, w0)
                                for kt in range(8):
                                    nc.tensor.matmul(pt, wqkv[:, kt, 256 + ct * P: 256 + (ct + 1) * P],
                                                     xc[:, kt, o0:o0 + w0],
                                                     start=(kt == 0), stop=(kt == 7))
                                nc.vector.tensor_mul(ksf[:, ct, sl], pt, ekf[:, sl])
                                nc.vector.tensor_mul(ksb[:, ct, sl], pt, ekb[:, sl])

                        # v projection (token-major)
                        vh = wrk.tile([P, NT7, HDV], bf16, tag="vh")
                        for tt in range(NT7):
                            tw = TW[tt]
                            pt = psum(tw, HDV)
                            for kt in range(8):
                                nc.tensor.matmul(pt, xc[:, kt, tt * P: tt * P + tw],
                                                 wqkv[:, kt, 512:1024],
                                                 start=(kt == 0), stop=(kt == 7))
                            nc.scalar.copy(vh[:tw, tt, :], pt)

                        # A + o per direction
                        for dr in range(2):
                            qs = qsf if dr == 0 else qsb
                            ks = ksf if dr == 0 else ksb
                            am = wrk.tile([P, NT7, L], bf16, tag="am")
                            for j in range(2):
                                jo, jw = ACH[j]
                                for si in range(NT7):
                                    d = si - 4 * j
                                    if dr == 0:
                                        if si * P > jo + jw - 1:
                                            continue        # fully masked
                                        mi_ = None if d < 0 else d
                                    else:
                                        if si * P + SW[si] - 1 < jo:
                                            continue
                                        mi_ = None if d >= 4 else 4 + d
                                    sw = SW[si]
                                    pt = psum(sw, jw)
                                    for ct in range(2):
                                        nc.tensor.matmul(pt, ks[:, ct, si * P: si * P + sw],
                                                         qs[:, ct, jo: jo + jw],
                                                         start=(ct == 0), stop=(ct == 1))
                                    if mi_ is None:
                                        nc.vector.tensor_copy(am[:sw, si, jo: jo + jw], pt)
                                    else:
                                        nc.vector.tensor_mul(am[:sw, si, jo: jo + jw], pt,
                                                             masks[:sw, mi_, :jw])

                            ssq = wrk.tile([P, 8], f32, tag="ssq", bufs=2)
                            nc.vector.memset(ssq[:], 0.0)
                            scrap = wrk.tile([P, HDV], bf16, tag="scrap")
                            for tt in range(NT7):
                                tw = TW[tt]
                                sis = list(range(0, tt + 1) if dr == 0 else range(tt, NT7))
                                pt = psum(tw, HDV)
                                for ii, si in enumerate(sis):
                                    nc.tensor.matmul(pt, am[:SW[si], si, tt * P: tt * P + tw],
                                                     vh[:SW[si], si, :],
                                                     start=(ii == 0), stop=(ii == len(sis) - 1))
                                nc.scalar.activation(scrap[:tw], pt, AF.Square,
                                                     accum_out=ssq[:tw, tt: tt + 1])
                                # rsl = (ssq/512 + eps)^-1/2 = exp(-0.5*ln(...))
                                rsl = wrk.tile([P, 1], f32, tag="rsl", bufs=2)
                                nc.scalar.activation(rsl[:tw], ssq[:tw, tt: tt + 1],
                                                     AF.Ln, scale=1.0 / HDV, bias=epst[:tw])
                                nc.scalar.activation(rsl[:tw], rsl[:tw], AF.Exp, scale=-0.5)
                                oslc = og[:tw, tt, h * HDV:(h + 1) * HDV]
                                if dr == 0:
                                    nc.vector.scalar_tensor_tensor(
                                        oslc, pt, rsl[:tw], zeros512[:tw], ALU.mult, ALU.add)
                                else:
                                    nc.vector.scalar_tensor_tensor(
                                        oslc, pt, rsl[:tw], oslc, ALU.mult, ALU.add)

                # ============ stage E: out = (ogT * gateF) @ ow ============
                owS0 = wst.tile([P, 16, 512], bf16, tag="wqkv", bufs=2, name="owS0")
                nc.sync.dma_start(
                    out=owS0, in_=ow_d[:, :, 0:512].rearrange("j p c -> p j c"))
                owS1 = wst.tile([P, 16, 512], bf16, tag="wqkv", bufs=2, name="owS1")
                nc.sync.dma_start(
                    out=owS1, in_=ow_d[:, :, 512:1024].rearrange("j p c -> p j c"))
                for tt in range(NT7):
                    tw = TW[tt]
                    ogT = est.tile([P, 16, P], bf16, tag="ogT")
                    for g in range(4):
                        ptT = ps.tile([P, 4, P], bf16, tag="psT", bufs=2, name="ptT")
                        for i in range(4):
                            jt = g * 4 + i
                            nc.tensor.transpose(ptT[:, i, :tw],
                                                og[:tw, tt, jt * P:(jt + 1) * P],
                                                ident[:tw, :tw])
                        nc.vector.tensor_mul(
                            ogT[:, g * 4:(g + 1) * 4, :tw],
                            ptT[:, :, :tw],
                            gateF[:, g * 4:(g + 1) * 4, tt * P: tt * P + tw])
                    for nch, owS in ((0, owS0), (1, owS1)):
                        pt = psum(tw, 512)
                        for jt in range(16):
                            nc.tensor.matmul(pt, ogT[:, jt, :tw], owS[:, jt, :],
                                             start=(jt == 0), stop=(jt == 15))
                        outs = sm.tile([P, 512], f32, tag="outs", bufs=2)
                        nc.vector.tensor_copy(outs[:tw], pt)
                        nc.sync.dma_start(
                            out=out_d[bi * L + tt * P: bi * L + tt * P + tw,
                                      nch * 512:(nch + 1) * 512],
                            in_=outs[:tw, :])

    _legalize_sync_waits(nc)
    return nc


_CACHE = {}


def _prep_shared(conv_w, qkv_w, gk_w1, gk_w2, gk_b2, g_w, o_w, gnorm_w, lnorm_w):
    bf = ml_dtypes.bfloat16
    cdg = np.zeros((9, 8, P, P), np.float32)
    w9 = conv_w.reshape(9, D)  # taps x channels (HWIO with I=1)
    idx = np.arange(P)
    for tap in range(9):
        for ft in range(8):
            cdg[tap, ft, idx, idx] = w9[tap, ft * P:(ft + 1) * P]
    assert np.allclose(gnorm_w, lnorm_w), "kernel assumes gnorm_w == lnorm_w (fold into o_w)"
    ow_eff = o_w * np.tile(gnorm_w, NH)[:, None]
    masks = np.zeros((8, P, 512), np.float32)
    s_i = np.arange(P)[:, None]
    t_i = np.arange(512)[None, :]
    for dd in range(4):
        masks[dd] = (s_i <= t_i - P * dd)
        masks[4 + dd] = (s_i >= t_i - P * dd)
    return {
        "cdg": np.ascontiguousarray(cdg.astype(bf)),
        "qkvw": np.ascontiguousarray(qkv_w.reshape(8, P, 4096).astype(bf)),
        "gk1w": np.ascontiguousarray(gk_w1.reshape(8, P, 16).astype(bf)),
        "gk2w": np.ascontiguousarray(gk_w2.astype(bf)),
        "b2": np.ascontiguousarray(gk_b2.reshape(16, P, 1).astype(np.float32)),
        "gw": np.ascontiguousarray(g_w.reshape(8, P, 2048).astype(bf)),
        "ow": np.ascontiguousarray(ow_eff.reshape(16, P, 1024).astype(bf)),
        "masks": np.ascontiguousarray(masks.astype(bf)),
        "ident": np.ascontiguousarray(np.eye(P, dtype=np.float32).astype(bf)),
    }


def kernel(x, conv_w, qkv_w, gk_w1, gk_w2, gk_b2, g_w, g_b, o_w, gnorm_w, lnorm_w, H, W,
           _return_res=False, _trace=False):
    x = np.asarray(x, np.float32)
    assert int(H) == 28 and int(W) == 28 and x.shape == (16, L, D)
    assert np.allclose(np.asarray(g_b), 0.0), "kernel assumes g_b == 0"
    bf = ml_dtypes.bfloat16

    if "nc" not in _CACHE:
        _CACHE["nc"] = _build_program()
    nc = _CACHE["nc"]

    shared = _prep_shared(np.asarray(conv_w, np.float32), np.asarray(qkv_w, np.float32),
                          np.asarray(gk_w1, np.float32), np.asarray(gk_w2, np.float32),
                          np.asarray(gk_b2, np.float32), np.asarray(g_w, np.float32),
                          np.asarray(o_w, np.float32), np.asarray(gnorm_w, np.float32),
                          np.asarray(lnorm_w, np.float32))
    in_maps = []
    for c in range(NCORES):
        xs = x[2 * c: 2 * c + 2]                       # [2, 784, 1024]
        xt = xs.reshape(B, 28, 28, D).transpose(3, 0, 1, 2)   # [1024, 2, 28, 28]
        xpad = np.zeros((D, B, 30, 30), np.float32)
        xpad[:, :, 1:29, 1:29] = xt
        m = dict(shared)
        m["xpad"] = np.ascontiguousarray(xpad.reshape(8, P, B * 900).astype(bf))
        in_maps.append(m)

    res = run_bass_kernel_spmd(nc, in_maps, core_ids=list(range(NCORES)), trace=_trace)
    out = np.concatenate([r["out"].reshape(B, L, D) for r in res.results], axis=0)
    if _return_res:
        return out, res
    return out
